# revision 45
# baseline (speedup 1.0000x reference)
"""GraphTransformerLayer on 8 TRN2 NeuronCores (Bass/Tile).

Sharding: query/node dim N=2048 split into 8 shards of 256 rows; each core
holds replicated K/V for all 2048 keys plus its 256-query shard.

Design (v3):
- All matmuls bf16 (fp32 runs at 1/4 PE rate); psum accumulation f32.
- Edge bias: dense per-core fp8 slab [16][128 keys, 8 heads, 256 queries],
  added into the score PSUM by seeding each accumulation group with an
  fp8 identity-matmul (start=True) before the K^T Q score matmuls
  (start=False, stop=True) accumulate on top.
- Heads are processed in the order [0,4,1,5,2,6,3,7] so that the two heads
  sharing one 2KB psum zero-region use the same PE quadrant row offset —
  mixing different sub-128 partition offsets in one zero region hangs the
  PE on hardware.
- Scores chunk-major: per key-chunk c (128 keys) and half g (4 heads), one
  [128, 4x256] psum tile -> one Exp activation psum->SBUF bf16 (pt).
  Unnormalized softmax; denominator comes from an all-ones column per head
  appended to V (attn @ [V|1]).
- attnV accumulates into memset-seeded persistent [128, 8, 33] psum tiles
  per query half (start=False throughout).
- V projection is spread across the chunk loop as PE filler; epilogue is
  stage-interleaved across the two query halves with activation-table
  switches batched (Exp -> Sqrt -> Gelu).
"""

import sys

sys.path.insert(0, "/opt/trn_rl_repo")

import numpy as np

import concourse.bacc as bacc
import concourse.mybir as mybir
import concourse.tile as tile
from concourse.bass_utils import run_bass_kernel_spmd

N_CORES = 8
N = 2048
D = 256
H = 8
DK = 32
QS = 256
H2 = 512
EPS = 1e-5
NCH = 16  # key chunks of 128

F32 = mybir.dt.float32
BF = mybir.dt.bfloat16
F8 = mybir.dt.float8e4

AF = mybir.ActivationFunctionType
ALU = mybir.AluOpType
AX = mybir.AxisListType

# Head processing order: pairs sharing a PE quadrant row offset (bp) share
# a psum zero region.
ORDER = [0, 4, 1, 5, 2, 6, 3, 7]


def build_kernel():
    nc = bacc.Bacc("TRN2", target_bir_lowering=False, debug=False,
                   num_devices=N_CORES)

    d_hT = nc.dram_tensor("hT", [D, N], BF, kind="ExternalInput")
    # qpack rows=feature dims; cols = [hT[:,0:256] | hTs | wq | wk | wv |
    # cols-as-bf16-bytes (24)]
    d_qpack = nc.dram_tensor("qpack", [D, 5 * D + 24], BF,
                             kind="ExternalInput")
    # epack row-blocks of 128; cols = [hres | wo | w1]
    d_epack = nc.dram_tensor("epack", [D, 4 * D], BF, kind="ExternalInput")
    # w2lng cols = [w2 blocks 0..3 | lng g1,b1,g2,b2]
    d_w2lng = nc.dram_tensor("w2lng", [128, 8 * D], BF, kind="ExternalInput")
    # idbo: [:,0:128]=identity bf16; row0 cols 128:384 = bo_eff
    d_idbo = nc.dram_tensor("idbo", [128, 384], BF, kind="ExternalInput")
    d_bias = nc.dram_tensor("bias8", [NCH, 128, H, QS], F8,
                            kind="ExternalInput")
    # b0pack: [:,0:256]=DoubleRow identity (fp8), [:,256:2304]=bias chunk 0
    d_b0 = nc.dram_tensor("b0pack", [128, 2304], F8, kind="ExternalInput")
    d_out = nc.dram_tensor("out", [QS, D], F32, kind="ExternalOutput")

    with tile.TileContext(nc) as tc:
        import contextlib

        with contextlib.ExitStack() as ctx:
            wp = ctx.enter_context(tc.tile_pool(name="w", bufs=1))
            bpool = ctx.enter_context(tc.tile_pool(name="bias", bufs=6))
            ptp = ctx.enter_context(tc.tile_pool(name="pt", bufs=6))
            sm = ctx.enter_context(tc.tile_pool(name="sm", bufs=2))
            ps_sc = ctx.enter_context(
                tc.tile_pool(name="psc", bufs=2, space="PSUM"))
            ps_at = ctx.enter_context(
                tc.tile_pool(name="pat", bufs=1, space="PSUM"))
            ps_ms = ctx.enter_context(
                tc.tile_pool(name="pms", bufs=2, space="PSUM"))

            def load(pool, dram, shape, name, dt, r0=0, c0=0):
                t = pool.tile(shape, dt, name=name, tag=name)
                nc.sync.dma_start(
                    t[:], dram.ap()[r0:r0 + shape[0], c0:c0 + shape[1]])
                return t

            # ---- critical-path loads first ----
            # hT loaded in column pieces so K-proj can start early
            hT = [wp.tile([128, N], BF, name=f"hT{i}", tag=f"hT{i}")
                  for i in range(2)]

            def load_hT_piece(c0, c1):
                for i in range(2):
                    nc.sync.dma_start(
                        hT[i][:, c0:c1],
                        d_hT.ap()[128 * i:128 * i + 128, c0:c1])

            bias_tiles = {}

            def emit_bias_dma(c):
                t = bpool.tile([128, H, QS], F8, tag="bias", name=f"bias{c}")
                nc.sync.dma_start(t[:], d_bias.ap()[c])
                bias_tiles[c] = t

            qpk = [load(wp, d_qpack, [128, 5 * D + 24], f"qpk{i}", BF,
                        128 * i) for i in range(2)]
            hT0 = [qpk[i][:, 0:D] for i in range(2)]
            hTs = [qpk[i][:, D:2 * D] for i in range(2)]
            wq = [qpk[i][:, 2 * D:3 * D] for i in range(2)]
            wk = [qpk[i][:, 3 * D:4 * D] for i in range(2)]
            wv = [qpk[i][:, 4 * D:5 * D] for i in range(2)]
            cols = qpk[0][:, 5 * D:5 * D + 24].bitcast(F32)
            b0t = wp.tile([128, 2304], F8, name="b0t", tag="b0t")
            nc.sync.dma_start(b0t[:], d_b0.ap()[:, :])
            id8 = b0t[:, 0:256].rearrange("p (a b) -> p a b", a=2)
            bias_tiles[0] = b0t[:, 256:2304].rearrange(
                "p (h q) -> p h q", h=H)
            load_hT_piece(256, 1024)
            emit_bias_dma(1)
            load_hT_piece(1024, 2048)
            emit_bias_dma(2)
            emit_bias_dma(3)

            bq = [cols[:, 0:1], cols[:, 1:2]]
            bk = [cols[:, 2:3], cols[:, 3:4]]
            b1c = [cols[:, 4 + i:5 + i] for i in range(4)]
            b2c = [cols[:, 8:9], cols[:, 9:10]]
            zcol = cols[:, 10:11]
            epscol = cols[:, 11:12]

            ones = wp.tile([1, 128], BF, name="ones", tag="ones")
            nc.vector.memset(ones[:], 1.0)

            att = [ps_at.tile([128, H, 33], F32, tag=f"att{qt}",
                              name=f"att{qt}") for qt in range(2)]

            # PE clock warm-up: dummy matmuls into the (not yet used) att
            # psum banks keep the PE busy from t=0 so its p-state ramps to
            # full clock before the real pipeline starts. att is memset
            # afterwards, before the attnV accumulation begins.
            def warm(n, qt=0):
                dst = att[qt][:].rearrange("p h e -> p (h e)")
                wrhs = ones[0:1, 0:1].broadcast_to((1, 264))
                for _ in range(n):
                    nc.tensor.matmul(dst, ones[:], wrhs,
                                     start=True, stop=True,
                                     skip_group_check=True)

            # V with per-head all-ones denominator column (col 32 of 33)
            v_sb = wp.tile([128, NCH, H, 33], BF, name="v_sb", tag="v_sb")
            nc.vector.memset(v_sb[:, :, :, 32:33], 1.0)

            # ---------- Q projection ----------
            warm(12, 0)
            QT = []
            for oc in range(2):
                ps = ps_ms.tile([128, 512], F32, tag="pms", name="psq")
                for ic in range(2):
                    nc.tensor.matmul(
                        ps[:, 0:QS], wq[ic][:, 128 * oc:128 * oc + 128],
                        hTs[ic][:], start=(ic == 0), stop=(ic == 1))
                t = wp.tile([128, QS], BF, name=f"QT{oc}", tag=f"QT{oc}")
                nc.vector.tensor_scalar_add(t[:], ps[:, 0:QS], bq[oc])
                QT.append(t)

            KT = [wp.tile([128, N], BF, name=f"KT{i}", tag=f"KT{i}")
                  for i in range(2)]
            pt_tiles = {}

            def emit_kproj(c0, c1):
                w = c1 - c0
                for oc in range(2):
                    ps = ps_ms.tile([128, 512], F32, tag="pms", name="psk")
                    for ic in range(2):
                        rhs = (hT0[ic][:, c0:c1] if c1 <= 256
                               else hT[ic][:, c0:c1])
                        nc.tensor.matmul(
                            ps[:, 0:w], wk[ic][:, 128 * oc:128 * oc + 128],
                            rhs, start=(ic == 0), stop=(ic == 1))
                    nc.vector.tensor_scalar_add(
                        KT[oc][:, c0:c1], ps[:, 0:w], bk[oc])

            def emit_vproj(c):
                ps = ps_ms.tile([128, 512], F32, tag="pms", name="psv")
                for ic in range(2):
                    lhs = (hT0[ic][:, 128 * c:128 * c + 128] if c < 2
                           else hT[ic][:, 128 * c:128 * c + 128])
                    nc.tensor.matmul(
                        ps[:, 0:D], lhs,
                        wv[ic][:], start=(ic == 0), stop=(ic == 1))
                nc.vector.tensor_copy(
                    v_sb[:, c, :, 0:32],
                    ps[:, 0:D].rearrange("p (h e) -> p h e", h=H))

            def emit_chunk(c):
                pt = ptp.tile([128, 2, 4, QS], BF, tag="pt", name=f"pt{c}")
                pt_tiles[c] = pt
                bt = bias_tiles.pop(c)
                for g in range(2):
                    ps = ps_sc.tile([128, 4, QS], F32, tag="sc",
                                    name=f"sc{c}_{g}")
                    for s in range(2):
                        rhs = bt[:, 4 * g + 2 * s:4 * g + 2 * s + 2, :]
                        rhs = rhs.rearrange("p a b -> p (a b)")
                        rhs = rhs.unsqueeze(1).broadcast_to((128, 2, 512))
                        nc.tensor.matmul(
                            ps[:, 2 * s:2 * s + 2, :], id8[:], rhs,
                            start=True, stop=False, skip_group_check=True,
                            perf_mode=mybir.MatmulPerfMode.DoubleRow)
                    for hh in range(4):
                        h = ORDER[4 * g + hh]
                        bp = 32 * (h % 4)
                        nc.tensor.matmul(
                            ps[:, hh, :],
                            KT[h // 4][bp:bp + 32, 128 * c:128 * c + 128],
                            QT[h // 4][bp:bp + 32, :],
                            start=False, stop=True, tile_position=(bp, 0),
                            skip_group_check=True)
                    nc.scalar.activation(pt[:, g], ps[:], AF.Exp, bias=zcol)

            def emit_attnv(c):
                pt = pt_tiles.pop(c)
                for qt in range(2):
                    for g in range(2):
                        for hh in range(4):
                            h = ORDER[4 * g + hh]
                            nc.tensor.matmul(
                                att[qt][:, h, :],
                                pt[:, g, hh, 128 * qt:128 * qt + 128],
                                v_sb[:, c, h, :],
                                start=False, stop=(c == NCH - 1),
                                skip_group_check=True)

            def emit_c(c):
                if c + 4 < NCH:
                    emit_bias_dma(c + 4)
                emit_vproj(c)
                emit_chunk(c)
                if c >= 2:
                    emit_attnv(c - 2)

            # ---------- main pipeline ----------
            warm(3, 1)
            for qt in range(2):
                nc.vector.memset(att[qt][:], 0.0)
            emit_kproj(0, 256)
            emit_c(0)
            emit_kproj(256, 512)
            emit_c(1)
            emit_kproj(512, 1024)
            emit_c(2)
            emit_c(3)
            emit_kproj(1024, 1536)
            emit_c(4)
            emit_c(5)
            emit_kproj(1536, 2048)

            # deferred (epilogue-only) loads
            epk = [load(wp, d_epack, [128, 4 * D], f"epk{i}", BF, 128 * i)
                   for i in range(2)]
            hres = [epk[i][:, 0:D] for i in range(2)]
            wo = [epk[i][:, D:2 * D] for i in range(2)]
            w1 = [epk[i][:, 2 * D:4 * D] for i in range(2)]
            w2l = load(wp, d_w2lng, [128, 8 * D], "w2l", BF)
            w2 = [w2l[:, D * i:D * (i + 1)] for i in range(4)]
            lng = w2l[:, 4 * D:8 * D]
            idbo = load(wp, d_idbo, [128, 384], "idbo", BF)
            idb = idbo[:, 0:128]
            bo = idbo[0:1, 128:384]

            for c in range(6, NCH):
                emit_c(c)
            emit_attnv(NCH - 2)
            emit_attnv(NCH - 1)

            # ---------- epilogue (stage-interleaved across qt) ----------
            o_nat = [wp.tile([128, D], BF, name=f"onat{qt}", tag=f"onat{qt}")
                     for qt in range(2)]
            OT = [wp.tile([128, D], BF, name=f"OT{fc}", tag=f"OT{fc}")
                  for fc in range(2)]
            fT = [wp.tile([128, D], BF, name=f"fT{fc}", tag=f"fT{fc}")
                  for fc in range(2)]
            g1T = [wp.tile([128, QS], BF, name=f"g1T{oc}", tag=f"g1T{oc}")
                   for oc in range(4)]
            y2T = [wp.tile([128, QS], BF, name=f"y2T{oc}", tag=f"y2T{oc}")
                   for oc in range(2)]
            h1 = [wp.tile([128, D], F32, name=f"h1_{qt}", tag=f"h1_{qt}")
                  for qt in range(2)]
            xin = [sm.tile([128, D], F32, tag=f"xin{qt}", name=f"xin{qt}")
                   for qt in range(2)]
            fln = [sm.tile([128, D], BF, tag=f"fln{qt}", name=f"fln{qt}")
                   for qt in range(2)]
            out_sb = [wp.tile([128, D], F32, name=f"osb{qt}", tag=f"osb{qt}")
                      for qt in range(2)]

            # normalize attention output (single broadcast multiply);
            # both reciprocals first so qt1 doesn't queue behind qt0's mult
            rdts = []
            for qt in range(2):
                rdt = sm.tile([128, H, 1], F32, tag=f"rd{qt}",
                              name=f"rd{qt}")
                nc.vector.reciprocal(rdt[:], att[qt][:, :, 32:33])
                rdts.append(rdt)
            for qt in range(2):
                nc.vector.tensor_mul(
                    o_nat[qt][:].rearrange("p (h e) -> p h e", h=H),
                    att[qt][:, :, 0:32],
                    rdts[qt][:].broadcast_to((128, H, 32)))
            def tr_ps(qt, name):
                if qt == 0:
                    return ps_ms.tile([128, 512], BF, tag="pms",
                                      name=name)[:, 0:128]
                return ps_sc.tile([128, 8, 128], BF, tag="sc",
                                  name=name)[:, 0, :]

            # transpose o_nat -> OT
            for qt in range(2):
                for fc in range(2):
                    tp = tr_ps(qt, "trp")
                    nc.tensor.transpose(
                        tp, o_nat[qt][:, 128 * fc:128 * fc + 128],
                        idb[:])
                    nc.vector.tensor_copy(
                        OT[fc][:, 128 * qt:128 * qt + 128], tp)
            # out-projection + residual
            for qt in range(2):
                if qt == 0:
                    aps = ps_ms.tile([128, 512], F32, tag="pms", name="apo")
                else:
                    aps = ps_sc.tile([128, 4, QS], F32, tag="sc",
                                     name="apo")[:, 0:2, :].rearrange(
                                         "p a b -> p (a b)")
                for ic in range(2):
                    nc.tensor.matmul(
                        aps[:, 0:D], OT[ic][:, 128 * qt:128 * qt + 128],
                        wo[ic][:], start=(ic == 0), stop=False)
                nc.tensor.matmul(aps[:, 0:D], ones[:], bo[:],
                                 start=False, stop=True)
                nc.vector.tensor_add(xin[qt][:], aps[:, 0:D], hres[qt][:])

            def ln_stats(x_ap, tagp, eng):
                """Emit LN stats for one tile; returns (x, nm, st)."""
                ss = sm.tile([128, 1], F32, tag=f"{tagp}ss", name="ss")
                nc.vector.reduce_sum(ss[:], x_ap, axis=AX.X)
                nm = sm.tile([128, 1], F32, tag=f"{tagp}nm", name="nm")
                eng.tensor_scalar_mul(nm[:], ss[:], -1.0 / D)
                scr = sm.tile([128, D], F32, tag=f"{tagp}scr", name="scr")
                vs = sm.tile([128, 1], F32, tag=f"{tagp}vs", name="vs")
                nc.scalar.activation(scr[:], x_ap, AF.Square, bias=nm[:],
                                     accum_out=vs[:])
                st = sm.tile([128, 1], F32, tag=f"{tagp}st", name="st")
                nc.scalar.activation(st[:], vs[:], AF.Sqrt, bias=epscol,
                                     scale=1.0 / D)
                return x_ap, nm, st

            def ln_apply(x_ap, nm, st, g_ap, b_ap, out_ap, tagp, eng):
                r0 = sm.tile([128, 1], F32, tag=f"{tagp}r0", name="r0")
                nc.vector.reciprocal(r0[:], st[:])
                if g_ap is None:  # gamma/beta folded into next matmul
                    eng.tensor_scalar(out_ap, x_ap, nm[:], r0[:],
                                      op0=ALU.add, op1=ALU.mult)
                    return
                t1 = sm.tile([128, D], F32, tag=f"{tagp}t1", name="t1")
                eng.scalar_tensor_tensor(t1[:], x_ap, nm[:], g_ap,
                                         ALU.add, ALU.mult)
                eng.scalar_tensor_tensor(out_ap, t1[:], r0[:], b_ap,
                                         ALU.mult, ALU.add)

            LNE = [nc.vector, nc.vector]

            # LN1 both qt (Sqrt ops batched; qt1 runs on gpsimd)
            st1 = []
            for qt in range(2):
                st1.append(ln_stats(xin[qt][:], f"l1q{qt}", LNE[qt]))
            for qt in range(2):
                x_ap, nm, st = st1[qt]
                ln_apply(x_ap, nm, st, lng[:, 0:D], lng[:, D:2 * D],
                         h1[qt][:], f"l1q{qt}", LNE[qt])
            # LN2 both qt
            st2 = []
            for qt in range(2):
                st2.append(ln_stats(h1[qt][:], f"l2q{qt}", LNE[qt]))
            for qt in range(2):
                x_ap, nm, st = st2[qt]
                ln_apply(x_ap, nm, st, None, None,
                         fln[qt][:], f"l2q{qt}", LNE[qt])
            # transpose fln -> fT
            for qt in range(2):
                for fc in range(2):
                    tp2 = tr_ps(qt, "trf")
                    nc.tensor.transpose(
                        tp2, fln[qt][:, 128 * fc:128 * fc + 128],
                        idb[:])
                    nc.vector.tensor_copy(
                        fT[fc][:, 128 * qt:128 * qt + 128], tp2)
            # FFN1 + gelu (gelus batched); psum from the now-idle score pool
            for qt in range(2):
                for oc in range(4):
                    fs = ps_sc.tile([128, 4, QS], F32, tag="sc", name="ps1")
                    fp = fs[:, 0, :]
                    for ic in range(2):
                        nc.tensor.matmul(
                            fp[:, 0:128],
                            w1[ic][:, 128 * oc:128 * oc + 128],
                            fT[ic][:, 128 * qt:128 * qt + 128],
                            start=(ic == 0), stop=(ic == 1))
                    nc.scalar.activation(
                        g1T[oc][:, 128 * qt:128 * qt + 128], fp[:, 0:128],
                        AF.Gelu, bias=b1c[oc])
            # FFN2 + transpose back + residual + store (per qt)
            for qt in range(2):
                for oc in range(2):
                    fs = ps_sc.tile([128, 4, QS], F32, tag="sc", name="ps2")
                    fp = fs[:, 0, :]
                    for ic in range(4):
                        nc.tensor.matmul(
                            fp[:, 0:128],
                            w2[ic][:, 128 * oc:128 * oc + 128],
                            g1T[ic][:, 128 * qt:128 * qt + 128],
                            start=(ic == 0), stop=(ic == 3))
                    nc.scalar.activation(
                        y2T[oc][:, 128 * qt:128 * qt + 128], fp[:, 0:128],
                        AF.Identity, bias=b2c[oc])
                for fc in range(2):
                    tp3 = tr_ps(qt, "trn")
                    nc.tensor.transpose(
                        tp3, y2T[fc][:, 128 * qt:128 * qt + 128],
                        idb[:])
                    nc.vector.tensor_add(
                        out_sb[qt][:, 128 * fc:128 * fc + 128],
                        h1[qt][:, 128 * fc:128 * fc + 128], tp3)
                nc.sync.dma_start(d_out.ap()[128 * qt:128 * qt + 128, :],
                                  out_sb[qt][:])

    nc.compile()
    return nc


_CACHE = {}
USE_FR = True


def _get_nc(use_fr=True):
    if use_fr not in _CACHE:
        _CACHE[use_fr] = build_kernel()
    return _CACHE[use_fr]


def kernel(**inputs):
    import ml_dtypes

    bf = ml_dtypes.bfloat16
    f8 = ml_dtypes.float8_e4m3fn

    h = np.asarray(inputs["h"], np.float32)
    edge_attr = np.asarray(inputs["edge_attr"], np.float32)
    edge_index = np.asarray(inputs["edge_index"])
    Wq, bq = np.asarray(inputs["Wq"], np.float32), np.asarray(inputs["bq"], np.float32)
    Wk, bk = np.asarray(inputs["Wk"], np.float32), np.asarray(inputs["bk"], np.float32)
    Wv, bv = np.asarray(inputs["Wv"], np.float32), np.asarray(inputs["bv"], np.float32)
    Wo, bo = np.asarray(inputs["Wo"], np.float32), np.asarray(inputs["bo"], np.float32)
    We, be = np.asarray(inputs["We"], np.float32), np.asarray(inputs["be"], np.float32)
    ln1_g, ln1_b = np.asarray(inputs["ln1_g"], np.float32), np.asarray(inputs["ln1_b"], np.float32)
    fln_g, fln_b = np.asarray(inputs["fln_g"], np.float32), np.asarray(inputs["fln_b"], np.float32)
    W1, b1 = np.asarray(inputs["W1"], np.float32), np.asarray(inputs["b1"], np.float32)
    W2, b2 = np.asarray(inputs["W2"], np.float32), np.asarray(inputs["b2"], np.float32)

    scale = 1.0 / np.sqrt(np.float32(DK))
    eb = edge_attr @ We + be  # (E, H)
    hT = np.ascontiguousarray(h.T)

    cols = np.zeros((128, 12), np.float32)
    cols[:, 0] = bq[0:128] * scale
    cols[:, 1] = bq[128:256] * scale
    cols[:, 2] = bk[0:128]
    cols[:, 3] = bk[128:256]
    b1_eff = b1 + fln_b @ W1
    for i in range(4):
        cols[:, 4 + i] = b1_eff[128 * i:128 * i + 128]
    cols[:, 8] = b2[0:128]
    cols[:, 9] = b2[128:256]
    cols[:, 11] = EPS

    bo_eff = (bv @ Wo + bo).reshape(1, D)
    lng = np.concatenate(
        [np.tile(x.reshape(1, D), (128, 1))
         for x in [ln1_g, ln1_b, fln_g, fln_b]], axis=1)

    w1_eff = fln_g.reshape(D, 1) * W1
    idbo = np.zeros((128, 384), np.float32)
    idbo[:, 0:128] = np.eye(128)
    idbo[0, 128:384] = bo_eff[0]
    w2lng = np.concatenate(
        [W2[0:128], W2[128:256], W2[256:384], W2[384:512], lng], axis=1)
    common = {
        "hT": hT.astype(bf),
        "w2lng": w2lng.astype(bf),
        "idbo": idbo.astype(bf),
    }
    id8dr = np.stack(
        [np.eye(128, dtype=np.float32),
         np.zeros((128, 128), np.float32)], axis=1).reshape(128, 256)
    wq_s = Wq * scale

    src = edge_index[0].astype(np.int64)
    dst = edge_index[1].astype(np.int64)
    in_maps = []
    for c in range(N_CORES):
        r0 = c * QS
        m = dict(common)
        qp = np.concatenate(
            [hT[:, 0:QS], hT[:, r0:r0 + QS], wq_s, Wk, Wv],
            axis=1).astype(bf)
        colsb = cols.astype(np.float32).view(bf).reshape(128, 24)
        m["qpack"] = np.ascontiguousarray(np.concatenate(
            [qp, np.concatenate([colsb, np.zeros_like(colsb)], axis=0)],
            axis=1))
        hres_c = h[r0:r0 + QS]
        m["epack"] = np.ascontiguousarray(np.concatenate(
            [np.concatenate([hres_c[128 * b:128 * b + 128],
                             Wo[128 * b:128 * b + 128],
                             w1_eff[128 * b:128 * b + 128]], axis=1)
             for b in range(2)], axis=0)).astype(bf)
        sel = (src >= r0) & (src < r0 + QS)
        slab = np.zeros((N, H, QS), np.float32)
        slab[dst[sel], :, src[sel] - r0] = eb[sel]
        slab = slab[:, ORDER, :]  # head slot order
        slab = slab.reshape(NCH, 128, H * QS)
        m["bias8"] = np.ascontiguousarray(slab).astype(f8).reshape(
            NCH, 128, H, QS)
        m["b0pack"] = np.ascontiguousarray(np.concatenate(
            [id8dr, slab[0]], axis=1)).astype(f8)
        in_maps.append(m)

    nc = _get_nc(USE_FR)
    res = run_bass_kernel_spmd(nc, in_maps, core_ids=list(range(N_CORES)))
    out = np.concatenate([res.results[cc]["out"] for cc in range(N_CORES)],
                         axis=0)
    return out.astype(np.float32)


# revision 46
# speedup vs baseline: 1.0019x; 1.0019x over previous
"""GraphTransformerLayer on 8 TRN2 NeuronCores (Bass/Tile).

Sharding: query/node dim N=2048 split into 8 shards of 256 rows; each core
holds replicated K/V for all 2048 keys plus its 256-query shard.

Design (v3):
- All matmuls bf16 (fp32 runs at 1/4 PE rate); psum accumulation f32.
- Edge bias: dense per-core fp8 slab [16][128 keys, 8 heads, 256 queries],
  added into the score PSUM by seeding each accumulation group with an
  fp8 identity-matmul (start=True) before the K^T Q score matmuls
  (start=False, stop=True) accumulate on top.
- Heads are processed in the order [0,4,1,5,2,6,3,7] so that the two heads
  sharing one 2KB psum zero-region use the same PE quadrant row offset —
  mixing different sub-128 partition offsets in one zero region hangs the
  PE on hardware.
- Scores chunk-major: per key-chunk c (128 keys) and half g (4 heads), one
  [128, 4x256] psum tile -> one Exp activation psum->SBUF bf16 (pt).
  Unnormalized softmax; denominator comes from an all-ones column per head
  appended to V (attn @ [V|1]).
- attnV accumulates into memset-seeded persistent [128, 8, 33] psum tiles
  per query half (start=False throughout).
- V projection is spread across the chunk loop as PE filler; epilogue is
  stage-interleaved across the two query halves with activation-table
  switches batched (Exp -> Sqrt -> Gelu).
"""

import sys

sys.path.insert(0, "/opt/trn_rl_repo")

import numpy as np

import concourse.bacc as bacc
import concourse.mybir as mybir
import concourse.tile as tile
from concourse.bass_utils import run_bass_kernel_spmd

N_CORES = 8
N = 2048
D = 256
H = 8
DK = 32
QS = 256
H2 = 512
EPS = 1e-5
NCH = 16  # key chunks of 128

F32 = mybir.dt.float32
BF = mybir.dt.bfloat16
F8 = mybir.dt.float8e4

AF = mybir.ActivationFunctionType
ALU = mybir.AluOpType
AX = mybir.AxisListType

# Head processing order: pairs sharing a PE quadrant row offset (bp) share
# a psum zero region.
ORDER = [0, 4, 1, 5, 2, 6, 3, 7]


def build_kernel():
    nc = bacc.Bacc("TRN2", target_bir_lowering=False, debug=False,
                   num_devices=N_CORES)

    d_hT = nc.dram_tensor("hT", [D, N], BF, kind="ExternalInput")
    # qpack rows=feature dims; cols = [hT[:,0:256] | hTs | wq | wk | wv |
    # cols-as-bf16-bytes (24)]
    d_qpack = nc.dram_tensor("qpack", [D, 5 * D + 24], BF,
                             kind="ExternalInput")
    # epack row-blocks of 128; cols = [hres | wo | w1]
    d_epack = nc.dram_tensor("epack", [D, 4 * D], BF, kind="ExternalInput")
    # w2lng cols = [w2 blocks 0..3 | lng g1,b1,g2,b2]
    d_w2lng = nc.dram_tensor("w2lng", [128, 8 * D], BF, kind="ExternalInput")
    # idbo: [:,0:128]=identity bf16; row0 cols 128:384 = bo_eff
    d_idbo = nc.dram_tensor("idbo", [128, 384], BF, kind="ExternalInput")
    d_bias = nc.dram_tensor("bias8", [NCH, 128, H, QS], F8,
                            kind="ExternalInput")
    # b0pack: [:,0:256]=DoubleRow identity (fp8), [:,256:2304]=bias chunk 0
    d_b0 = nc.dram_tensor("b0pack", [128, 2304], F8, kind="ExternalInput")
    d_out = nc.dram_tensor("out", [QS, D], F32, kind="ExternalOutput")

    with tile.TileContext(nc) as tc:
        import contextlib

        with contextlib.ExitStack() as ctx:
            wp = ctx.enter_context(tc.tile_pool(name="w", bufs=1))
            bpool = ctx.enter_context(tc.tile_pool(name="bias", bufs=6))
            ptp = ctx.enter_context(tc.tile_pool(name="pt", bufs=6))
            sm = ctx.enter_context(tc.tile_pool(name="sm", bufs=2))
            ps_sc = ctx.enter_context(
                tc.tile_pool(name="psc", bufs=2, space="PSUM"))
            ps_at = ctx.enter_context(
                tc.tile_pool(name="pat", bufs=1, space="PSUM"))
            ps_ms = ctx.enter_context(
                tc.tile_pool(name="pms", bufs=2, space="PSUM"))

            def load(pool, dram, shape, name, dt, r0=0, c0=0):
                t = pool.tile(shape, dt, name=name, tag=name)
                nc.sync.dma_start(
                    t[:], dram.ap()[r0:r0 + shape[0], c0:c0 + shape[1]])
                return t

            # ---- critical-path loads first ----
            # hT loaded in column pieces so K-proj can start early
            hT = [wp.tile([128, N], BF, name=f"hT{i}", tag=f"hT{i}")
                  for i in range(2)]

            def load_hT_piece(c0, c1):
                for i in range(2):
                    nc.sync.dma_start(
                        hT[i][:, c0:c1],
                        d_hT.ap()[128 * i:128 * i + 128, c0:c1])

            bias_tiles = {}

            def emit_bias_dma(c):
                t = bpool.tile([128, H, QS], F8, tag="bias", name=f"bias{c}")
                nc.sync.dma_start(t[:], d_bias.ap()[c])
                bias_tiles[c] = t

            qpk = [load(wp, d_qpack, [128, 5 * D + 24], f"qpk{i}", BF,
                        128 * i) for i in range(2)]
            hT0 = [qpk[i][:, 0:D] for i in range(2)]
            hTs = [qpk[i][:, D:2 * D] for i in range(2)]
            wq = [qpk[i][:, 2 * D:3 * D] for i in range(2)]
            wk = [qpk[i][:, 3 * D:4 * D] for i in range(2)]
            wv = [qpk[i][:, 4 * D:5 * D] for i in range(2)]
            cols = qpk[0][:, 5 * D:5 * D + 24].bitcast(F32)
            b0t = wp.tile([128, 2304], F8, name="b0t", tag="b0t")
            nc.sync.dma_start(b0t[:], d_b0.ap()[:, :])
            id8 = b0t[:, 0:256].rearrange("p (a b) -> p a b", a=2)
            bias_tiles[0] = b0t[:, 256:2304].rearrange(
                "p (h q) -> p h q", h=H)
            load_hT_piece(256, 1024)
            emit_bias_dma(1)
            load_hT_piece(1024, 2048)
            emit_bias_dma(2)
            emit_bias_dma(3)

            bq = [cols[:, 0:1], cols[:, 1:2]]
            bk = [cols[:, 2:3], cols[:, 3:4]]
            b1c = [cols[:, 4 + i:5 + i] for i in range(4)]
            b2c = [cols[:, 8:9], cols[:, 9:10]]
            zcol = cols[:, 10:11]
            epscol = cols[:, 11:12]

            ones = wp.tile([1, 128], BF, name="ones", tag="ones")
            nc.vector.memset(ones[:], 1.0)

            att = [ps_at.tile([128, H, 33], F32, tag=f"att{qt}",
                              name=f"att{qt}") for qt in range(2)]

            # PE clock warm-up: dummy matmuls into the (not yet used) att
            # psum banks keep the PE busy from t=0 so its p-state ramps to
            # full clock before the real pipeline starts. att is memset
            # afterwards, before the attnV accumulation begins.
            def warm(n, qt=0):
                dst = att[qt][:].rearrange("p h e -> p (h e)")
                wrhs = ones[0:1, 0:1].broadcast_to((1, 264))
                for _ in range(n):
                    nc.tensor.matmul(dst, ones[:], wrhs,
                                     start=True, stop=True,
                                     skip_group_check=True)

            # V with per-head all-ones denominator column (col 32 of 33)
            v_sb = wp.tile([128, NCH, H, 33], BF, name="v_sb", tag="v_sb")
            nc.vector.memset(v_sb[:, :, :, 32:33], 1.0)

            # ---------- Q projection ----------
            warm(8, 0)
            QT = []
            for oc in range(2):
                ps = ps_ms.tile([128, 512], F32, tag="pms", name="psq")
                for ic in range(2):
                    nc.tensor.matmul(
                        ps[:, 0:QS], wq[ic][:, 128 * oc:128 * oc + 128],
                        hTs[ic][:], start=(ic == 0), stop=(ic == 1))
                t = wp.tile([128, QS], BF, name=f"QT{oc}", tag=f"QT{oc}")
                nc.vector.tensor_scalar_add(t[:], ps[:, 0:QS], bq[oc])
                QT.append(t)

            KT = [wp.tile([128, N], BF, name=f"KT{i}", tag=f"KT{i}")
                  for i in range(2)]
            pt_tiles = {}

            def emit_kproj(c0, c1):
                w = c1 - c0
                for oc in range(2):
                    ps = ps_ms.tile([128, 512], F32, tag="pms", name="psk")
                    for ic in range(2):
                        rhs = (hT0[ic][:, c0:c1] if c1 <= 256
                               else hT[ic][:, c0:c1])
                        nc.tensor.matmul(
                            ps[:, 0:w], wk[ic][:, 128 * oc:128 * oc + 128],
                            rhs, start=(ic == 0), stop=(ic == 1))
                    nc.vector.tensor_scalar_add(
                        KT[oc][:, c0:c1], ps[:, 0:w], bk[oc])

            def emit_vproj(c):
                ps = ps_ms.tile([128, 512], F32, tag="pms", name="psv")
                for ic in range(2):
                    lhs = (hT0[ic][:, 128 * c:128 * c + 128] if c < 2
                           else hT[ic][:, 128 * c:128 * c + 128])
                    nc.tensor.matmul(
                        ps[:, 0:D], lhs,
                        wv[ic][:], start=(ic == 0), stop=(ic == 1))
                nc.vector.tensor_copy(
                    v_sb[:, c, :, 0:32],
                    ps[:, 0:D].rearrange("p (h e) -> p h e", h=H))

            def emit_chunk(c):
                pt = ptp.tile([128, 2, 4, QS], BF, tag="pt", name=f"pt{c}")
                pt_tiles[c] = pt
                bt = bias_tiles.pop(c)
                for g in range(2):
                    ps = ps_sc.tile([128, 4, QS], F32, tag="sc",
                                    name=f"sc{c}_{g}")
                    for s in range(2):
                        rhs = bt[:, 4 * g + 2 * s:4 * g + 2 * s + 2, :]
                        rhs = rhs.rearrange("p a b -> p (a b)")
                        rhs = rhs.unsqueeze(1).broadcast_to((128, 2, 512))
                        nc.tensor.matmul(
                            ps[:, 2 * s:2 * s + 2, :], id8[:], rhs,
                            start=True, stop=False, skip_group_check=True,
                            perf_mode=mybir.MatmulPerfMode.DoubleRow)
                    for hh in range(4):
                        h = ORDER[4 * g + hh]
                        bp = 32 * (h % 4)
                        nc.tensor.matmul(
                            ps[:, hh, :],
                            KT[h // 4][bp:bp + 32, 128 * c:128 * c + 128],
                            QT[h // 4][bp:bp + 32, :],
                            start=False, stop=True, tile_position=(bp, 0),
                            skip_group_check=True)
                    nc.scalar.activation(pt[:, g], ps[:], AF.Exp, bias=zcol)

            def emit_attnv(c):
                pt = pt_tiles.pop(c)
                for qt in range(2):
                    for g in range(2):
                        for hh in range(4):
                            h = ORDER[4 * g + hh]
                            nc.tensor.matmul(
                                att[qt][:, h, :],
                                pt[:, g, hh, 128 * qt:128 * qt + 128],
                                v_sb[:, c, h, :],
                                start=False, stop=(c == NCH - 1),
                                skip_group_check=True)

            def emit_c(c):
                if c + 4 < NCH:
                    emit_bias_dma(c + 4)
                emit_vproj(c)
                emit_chunk(c)
                if c >= 2:
                    emit_attnv(c - 2)

            # ---------- main pipeline ----------
            warm(3, 1)
            for qt in range(2):
                nc.vector.memset(att[qt][:], 0.0)
            emit_kproj(0, 256)
            emit_c(0)
            emit_kproj(256, 512)
            emit_c(1)
            emit_kproj(512, 1024)
            emit_c(2)
            emit_c(3)
            emit_kproj(1024, 1536)
            emit_c(4)
            emit_c(5)
            emit_kproj(1536, 2048)

            # deferred (epilogue-only) loads
            epk = [load(wp, d_epack, [128, 4 * D], f"epk{i}", BF, 128 * i)
                   for i in range(2)]
            hres = [epk[i][:, 0:D] for i in range(2)]
            wo = [epk[i][:, D:2 * D] for i in range(2)]
            w1 = [epk[i][:, 2 * D:4 * D] for i in range(2)]
            w2l = load(wp, d_w2lng, [128, 8 * D], "w2l", BF)
            w2 = [w2l[:, D * i:D * (i + 1)] for i in range(4)]
            lng = w2l[:, 4 * D:8 * D]
            idbo = load(wp, d_idbo, [128, 384], "idbo", BF)
            idb = idbo[:, 0:128]
            bo = idbo[0:1, 128:384]

            for c in range(6, NCH):
                emit_c(c)
            emit_attnv(NCH - 2)
            emit_attnv(NCH - 1)

            # ---------- epilogue (stage-interleaved across qt) ----------
            o_nat = [wp.tile([128, D], BF, name=f"onat{qt}", tag=f"onat{qt}")
                     for qt in range(2)]
            OT = [wp.tile([128, D], BF, name=f"OT{fc}", tag=f"OT{fc}")
                  for fc in range(2)]
            fT = [wp.tile([128, D], BF, name=f"fT{fc}", tag=f"fT{fc}")
                  for fc in range(2)]
            g1T = [wp.tile([128, QS], BF, name=f"g1T{oc}", tag=f"g1T{oc}")
                   for oc in range(4)]
            y2T = [wp.tile([128, QS], BF, name=f"y2T{oc}", tag=f"y2T{oc}")
                   for oc in range(2)]
            h1 = [wp.tile([128, D], F32, name=f"h1_{qt}", tag=f"h1_{qt}")
                  for qt in range(2)]
            xin = [sm.tile([128, D], F32, tag=f"xin{qt}", name=f"xin{qt}")
                   for qt in range(2)]
            fln = [sm.tile([128, D], BF, tag=f"fln{qt}", name=f"fln{qt}")
                   for qt in range(2)]
            out_sb = [wp.tile([128, D], F32, name=f"osb{qt}", tag=f"osb{qt}")
                      for qt in range(2)]

            # normalize attention output (single broadcast multiply);
            # both reciprocals first so qt1 doesn't queue behind qt0's mult
            rdts = []
            for qt in range(2):
                rdt = sm.tile([128, H, 1], F32, tag=f"rd{qt}",
                              name=f"rd{qt}")
                nc.vector.reciprocal(rdt[:], att[qt][:, :, 32:33])
                rdts.append(rdt)
            for qt in range(2):
                nc.vector.tensor_mul(
                    o_nat[qt][:].rearrange("p (h e) -> p h e", h=H),
                    att[qt][:, :, 0:32],
                    rdts[qt][:].broadcast_to((128, H, 32)))
            def tr_ps(qt, name):
                if qt == 0:
                    return ps_ms.tile([128, 512], BF, tag="pms",
                                      name=name)[:, 0:128]
                return ps_sc.tile([128, 8, 128], BF, tag="sc",
                                  name=name)[:, 0, :]

            # transpose o_nat -> OT
            for qt in range(2):
                for fc in range(2):
                    tp = tr_ps(qt, "trp")
                    nc.tensor.transpose(
                        tp, o_nat[qt][:, 128 * fc:128 * fc + 128],
                        idb[:])
                    nc.vector.tensor_copy(
                        OT[fc][:, 128 * qt:128 * qt + 128], tp)
            # out-projection + residual
            for qt in range(2):
                if qt == 0:
                    aps = ps_ms.tile([128, 512], F32, tag="pms", name="apo")
                else:
                    aps = ps_sc.tile([128, 4, QS], F32, tag="sc",
                                     name="apo")[:, 0:2, :].rearrange(
                                         "p a b -> p (a b)")
                for ic in range(2):
                    nc.tensor.matmul(
                        aps[:, 0:D], OT[ic][:, 128 * qt:128 * qt + 128],
                        wo[ic][:], start=(ic == 0), stop=False)
                nc.tensor.matmul(aps[:, 0:D], ones[:], bo[:],
                                 start=False, stop=True)
                nc.vector.tensor_add(xin[qt][:], aps[:, 0:D], hres[qt][:])

            def ln_stats(x_ap, tagp, eng):
                """Emit LN stats for one tile; returns (x, nm, st)."""
                ss = sm.tile([128, 1], F32, tag=f"{tagp}ss", name="ss")
                nc.vector.reduce_sum(ss[:], x_ap, axis=AX.X)
                nm = sm.tile([128, 1], F32, tag=f"{tagp}nm", name="nm")
                eng.tensor_scalar_mul(nm[:], ss[:], -1.0 / D)
                scr = sm.tile([128, D], F32, tag=f"{tagp}scr", name="scr")
                vs = sm.tile([128, 1], F32, tag=f"{tagp}vs", name="vs")
                nc.scalar.activation(scr[:], x_ap, AF.Square, bias=nm[:],
                                     accum_out=vs[:])
                st = sm.tile([128, 1], F32, tag=f"{tagp}st", name="st")
                nc.scalar.activation(st[:], vs[:], AF.Sqrt, bias=epscol,
                                     scale=1.0 / D)
                return x_ap, nm, st

            def ln_apply(x_ap, nm, st, g_ap, b_ap, out_ap, tagp, eng):
                r0 = sm.tile([128, 1], F32, tag=f"{tagp}r0", name="r0")
                nc.vector.reciprocal(r0[:], st[:])
                if g_ap is None:  # gamma/beta folded into next matmul
                    eng.tensor_scalar(out_ap, x_ap, nm[:], r0[:],
                                      op0=ALU.add, op1=ALU.mult)
                    return
                t1 = sm.tile([128, D], F32, tag=f"{tagp}t1", name="t1")
                eng.scalar_tensor_tensor(t1[:], x_ap, nm[:], g_ap,
                                         ALU.add, ALU.mult)
                eng.scalar_tensor_tensor(out_ap, t1[:], r0[:], b_ap,
                                         ALU.mult, ALU.add)

            LNE = [nc.vector, nc.vector]

            # LN1 both qt (Sqrt ops batched; qt1 runs on gpsimd)
            st1 = []
            for qt in range(2):
                st1.append(ln_stats(xin[qt][:], f"l1q{qt}", LNE[qt]))
            for qt in range(2):
                x_ap, nm, st = st1[qt]
                ln_apply(x_ap, nm, st, lng[:, 0:D], lng[:, D:2 * D],
                         h1[qt][:], f"l1q{qt}", LNE[qt])
            # LN2 both qt
            st2 = []
            for qt in range(2):
                st2.append(ln_stats(h1[qt][:], f"l2q{qt}", LNE[qt]))
            for qt in range(2):
                x_ap, nm, st = st2[qt]
                ln_apply(x_ap, nm, st, None, None,
                         fln[qt][:], f"l2q{qt}", LNE[qt])
            # transpose fln -> fT
            for qt in range(2):
                for fc in range(2):
                    tp2 = tr_ps(qt, "trf")
                    nc.tensor.transpose(
                        tp2, fln[qt][:, 128 * fc:128 * fc + 128],
                        idb[:])
                    nc.vector.tensor_copy(
                        fT[fc][:, 128 * qt:128 * qt + 128], tp2)
            # FFN1 + gelu (gelus batched); psum from the now-idle score pool
            for qt in range(2):
                for oc in range(4):
                    fs = ps_sc.tile([128, 4, QS], F32, tag="sc", name="ps1")
                    fp = fs[:, 0, :]
                    for ic in range(2):
                        nc.tensor.matmul(
                            fp[:, 0:128],
                            w1[ic][:, 128 * oc:128 * oc + 128],
                            fT[ic][:, 128 * qt:128 * qt + 128],
                            start=(ic == 0), stop=(ic == 1))
                    nc.scalar.activation(
                        g1T[oc][:, 128 * qt:128 * qt + 128], fp[:, 0:128],
                        AF.Gelu, bias=b1c[oc])
            # FFN2 + transpose back + residual + store (per qt)
            for qt in range(2):
                for oc in range(2):
                    fs = ps_sc.tile([128, 4, QS], F32, tag="sc", name="ps2")
                    fp = fs[:, 0, :]
                    for ic in range(4):
                        nc.tensor.matmul(
                            fp[:, 0:128],
                            w2[ic][:, 128 * oc:128 * oc + 128],
                            g1T[ic][:, 128 * qt:128 * qt + 128],
                            start=(ic == 0), stop=(ic == 3))
                    nc.scalar.activation(
                        y2T[oc][:, 128 * qt:128 * qt + 128], fp[:, 0:128],
                        AF.Identity, bias=b2c[oc])
                for fc in range(2):
                    tp3 = tr_ps(qt, "trn")
                    nc.tensor.transpose(
                        tp3, y2T[fc][:, 128 * qt:128 * qt + 128],
                        idb[:])
                    nc.vector.tensor_add(
                        out_sb[qt][:, 128 * fc:128 * fc + 128],
                        h1[qt][:, 128 * fc:128 * fc + 128], tp3)
                nc.sync.dma_start(d_out.ap()[128 * qt:128 * qt + 128, :],
                                  out_sb[qt][:])

    nc.compile()
    return nc


_CACHE = {}
USE_FR = True


def _get_nc(use_fr=True):
    if use_fr not in _CACHE:
        _CACHE[use_fr] = build_kernel()
    return _CACHE[use_fr]


def kernel(**inputs):
    import ml_dtypes

    bf = ml_dtypes.bfloat16
    f8 = ml_dtypes.float8_e4m3fn

    h = np.asarray(inputs["h"], np.float32)
    edge_attr = np.asarray(inputs["edge_attr"], np.float32)
    edge_index = np.asarray(inputs["edge_index"])
    Wq, bq = np.asarray(inputs["Wq"], np.float32), np.asarray(inputs["bq"], np.float32)
    Wk, bk = np.asarray(inputs["Wk"], np.float32), np.asarray(inputs["bk"], np.float32)
    Wv, bv = np.asarray(inputs["Wv"], np.float32), np.asarray(inputs["bv"], np.float32)
    Wo, bo = np.asarray(inputs["Wo"], np.float32), np.asarray(inputs["bo"], np.float32)
    We, be = np.asarray(inputs["We"], np.float32), np.asarray(inputs["be"], np.float32)
    ln1_g, ln1_b = np.asarray(inputs["ln1_g"], np.float32), np.asarray(inputs["ln1_b"], np.float32)
    fln_g, fln_b = np.asarray(inputs["fln_g"], np.float32), np.asarray(inputs["fln_b"], np.float32)
    W1, b1 = np.asarray(inputs["W1"], np.float32), np.asarray(inputs["b1"], np.float32)
    W2, b2 = np.asarray(inputs["W2"], np.float32), np.asarray(inputs["b2"], np.float32)

    scale = 1.0 / np.sqrt(np.float32(DK))
    eb = edge_attr @ We + be  # (E, H)
    hT = np.ascontiguousarray(h.T)

    cols = np.zeros((128, 12), np.float32)
    cols[:, 0] = bq[0:128] * scale
    cols[:, 1] = bq[128:256] * scale
    cols[:, 2] = bk[0:128]
    cols[:, 3] = bk[128:256]
    b1_eff = b1 + fln_b @ W1
    for i in range(4):
        cols[:, 4 + i] = b1_eff[128 * i:128 * i + 128]
    cols[:, 8] = b2[0:128]
    cols[:, 9] = b2[128:256]
    cols[:, 11] = EPS

    bo_eff = (bv @ Wo + bo).reshape(1, D)
    lng = np.concatenate(
        [np.tile(x.reshape(1, D), (128, 1))
         for x in [ln1_g, ln1_b, fln_g, fln_b]], axis=1)

    w1_eff = fln_g.reshape(D, 1) * W1
    idbo = np.zeros((128, 384), np.float32)
    idbo[:, 0:128] = np.eye(128)
    idbo[0, 128:384] = bo_eff[0]
    w2lng = np.concatenate(
        [W2[0:128], W2[128:256], W2[256:384], W2[384:512], lng], axis=1)
    common = {
        "hT": hT.astype(bf),
        "w2lng": w2lng.astype(bf),
        "idbo": idbo.astype(bf),
    }
    id8dr = np.stack(
        [np.eye(128, dtype=np.float32),
         np.zeros((128, 128), np.float32)], axis=1).reshape(128, 256)
    wq_s = Wq * scale

    src = edge_index[0].astype(np.int64)
    dst = edge_index[1].astype(np.int64)
    in_maps = []
    for c in range(N_CORES):
        r0 = c * QS
        m = dict(common)
        qp = np.concatenate(
            [hT[:, 0:QS], hT[:, r0:r0 + QS], wq_s, Wk, Wv],
            axis=1).astype(bf)
        colsb = cols.astype(np.float32).view(bf).reshape(128, 24)
        m["qpack"] = np.ascontiguousarray(np.concatenate(
            [qp, np.concatenate([colsb, np.zeros_like(colsb)], axis=0)],
            axis=1))
        hres_c = h[r0:r0 + QS]
        m["epack"] = np.ascontiguousarray(np.concatenate(
            [np.concatenate([hres_c[128 * b:128 * b + 128],
                             Wo[128 * b:128 * b + 128],
                             w1_eff[128 * b:128 * b + 128]], axis=1)
             for b in range(2)], axis=0)).astype(bf)
        sel = (src >= r0) & (src < r0 + QS)
        slab = np.zeros((N, H, QS), np.float32)
        slab[dst[sel], :, src[sel] - r0] = eb[sel]
        slab = slab[:, ORDER, :]  # head slot order
        slab = slab.reshape(NCH, 128, H * QS)
        m["bias8"] = np.ascontiguousarray(slab).astype(f8).reshape(
            NCH, 128, H, QS)
        m["b0pack"] = np.ascontiguousarray(np.concatenate(
            [id8dr, slab[0]], axis=1)).astype(f8)
        in_maps.append(m)

    nc = _get_nc(USE_FR)
    res = run_bass_kernel_spmd(nc, in_maps, core_ids=list(range(N_CORES)))
    out = np.concatenate([res.results[cc]["out"] for cc in range(N_CORES)],
                         axis=0)
    return out.astype(np.float32)


# revision 47
# speedup vs baseline: 1.0120x; 1.0100x over previous
"""GraphTransformerLayer on 8 TRN2 NeuronCores (Bass/Tile).

Sharding: query/node dim N=2048 split into 8 shards of 256 rows; each core
holds replicated K/V for all 2048 keys plus its 256-query shard.

Design (v3):
- All matmuls bf16 (fp32 runs at 1/4 PE rate); psum accumulation f32.
- Edge bias: dense per-core fp8 slab [16][128 keys, 8 heads, 256 queries],
  added into the score PSUM by seeding each accumulation group with an
  fp8 identity-matmul (start=True) before the K^T Q score matmuls
  (start=False, stop=True) accumulate on top.
- Heads are processed in the order [0,4,1,5,2,6,3,7] so that the two heads
  sharing one 2KB psum zero-region use the same PE quadrant row offset —
  mixing different sub-128 partition offsets in one zero region hangs the
  PE on hardware.
- Scores chunk-major: per key-chunk c (128 keys) and half g (4 heads), one
  [128, 4x256] psum tile -> one Exp activation psum->SBUF bf16 (pt).
  Unnormalized softmax; denominator comes from an all-ones column per head
  appended to V (attn @ [V|1]).
- attnV accumulates into memset-seeded persistent [128, 8, 33] psum tiles
  per query half (start=False throughout).
- V projection is spread across the chunk loop as PE filler; epilogue is
  stage-interleaved across the two query halves with activation-table
  switches batched (Exp -> Sqrt -> Gelu).
"""

import sys

sys.path.insert(0, "/opt/trn_rl_repo")

import numpy as np

import concourse.bacc as bacc
import concourse.mybir as mybir
import concourse.tile as tile
from concourse.bass_utils import run_bass_kernel_spmd

N_CORES = 8
N = 2048
D = 256
H = 8
DK = 32
QS = 256
H2 = 512
EPS = 1e-5
NCH = 16  # key chunks of 128

F32 = mybir.dt.float32
BF = mybir.dt.bfloat16
F8 = mybir.dt.float8e4

AF = mybir.ActivationFunctionType
ALU = mybir.AluOpType
AX = mybir.AxisListType

# Head processing order: pairs sharing a PE quadrant row offset (bp) share
# a psum zero region.
ORDER = [0, 4, 1, 5, 2, 6, 3, 7]


def build_kernel():
    nc = bacc.Bacc("TRN2", target_bir_lowering=False, debug=False,
                   num_devices=N_CORES)

    d_hT = nc.dram_tensor("hT", [D, N], BF, kind="ExternalInput")
    # qpack rows=feature dims; cols = [hT[:,0:256] | hTs | wq | wk | wv |
    # cols-as-bf16-bytes (24)]
    d_qpack = nc.dram_tensor("qpack", [D, 5 * D + 32], BF,
                             kind="ExternalInput")
    # epack row-blocks of 128; cols = [hres | wo | w1]
    d_epack = nc.dram_tensor("epack", [D, 4 * D], BF, kind="ExternalInput")
    # w2lng cols = [w2 blocks 0..3 | lng g1,b1,g2,b2]
    d_w2lng = nc.dram_tensor("w2lng", [128, 8 * D], BF, kind="ExternalInput")
    # idbo: [:,0:128]=identity bf16; row0 cols 128:384 = bo_eff
    d_idbo = nc.dram_tensor("idbo", [128, 384], BF, kind="ExternalInput")
    d_bias = nc.dram_tensor("bias8", [NCH, 128, H, QS], F8,
                            kind="ExternalInput")
    # b0pack: [:,0:256]=DoubleRow identity (fp8), [:,256:2304]=bias chunk 0
    d_b0 = nc.dram_tensor("b0pack", [128, 2304], F8, kind="ExternalInput")
    d_out = nc.dram_tensor("out", [QS, D], F32, kind="ExternalOutput")

    with tile.TileContext(nc) as tc:
        import contextlib

        with contextlib.ExitStack() as ctx:
            wp = ctx.enter_context(tc.tile_pool(name="w", bufs=1))
            bpool = ctx.enter_context(tc.tile_pool(name="bias", bufs=6))
            ptp = ctx.enter_context(tc.tile_pool(name="pt", bufs=6))
            sm = ctx.enter_context(tc.tile_pool(name="sm", bufs=2))
            ps_sc = ctx.enter_context(
                tc.tile_pool(name="psc", bufs=2, space="PSUM"))
            ps_at = ctx.enter_context(
                tc.tile_pool(name="pat", bufs=1, space="PSUM"))
            ps_ms = ctx.enter_context(
                tc.tile_pool(name="pms", bufs=2, space="PSUM"))

            def load(pool, dram, shape, name, dt, r0=0, c0=0):
                t = pool.tile(shape, dt, name=name, tag=name)
                nc.sync.dma_start(
                    t[:], dram.ap()[r0:r0 + shape[0], c0:c0 + shape[1]])
                return t

            # ---- critical-path loads first ----
            # hT loaded in column pieces so K-proj can start early
            hT = [wp.tile([128, N], BF, name=f"hT{i}", tag=f"hT{i}")
                  for i in range(2)]

            def load_hT_piece(c0, c1):
                for i in range(2):
                    nc.sync.dma_start(
                        hT[i][:, c0:c1],
                        d_hT.ap()[128 * i:128 * i + 128, c0:c1])

            bias_tiles = {}

            def emit_bias_dma(c):
                t = bpool.tile([128, H, QS], F8, tag="bias", name=f"bias{c}")
                nc.sync.dma_start(t[:], d_bias.ap()[c])
                bias_tiles[c] = t

            qpk = [load(wp, d_qpack, [128, 5 * D + 32], f"qpk{i}", BF,
                        128 * i) for i in range(2)]
            hT0 = [qpk[i][:, 0:D] for i in range(2)]
            hTs = [qpk[i][:, D:2 * D] for i in range(2)]
            wq = [qpk[i][:, 2 * D:3 * D] for i in range(2)]
            wk = [qpk[i][:, 3 * D:4 * D] for i in range(2)]
            wv = [qpk[i][:, 4 * D:5 * D] for i in range(2)]
            cols = qpk[0][:, 5 * D:5 * D + 32].bitcast(F32)
            b0t = wp.tile([128, 2304], F8, name="b0t", tag="b0t")
            nc.sync.dma_start(b0t[:], d_b0.ap()[:, :])
            id8 = b0t[:, 0:256].rearrange("p (a b) -> p a b", a=2)
            bias_tiles[0] = b0t[:, 256:2304].rearrange(
                "p (h q) -> p h q", h=H)
            load_hT_piece(256, 1024)
            emit_bias_dma(1)
            load_hT_piece(1024, 2048)
            emit_bias_dma(2)
            emit_bias_dma(3)

            bq = [cols[:, 0:1], cols[:, 1:2]]
            bk = [cols[:, 2:3], cols[:, 3:4]]
            b1c = [cols[:, 4 + i:5 + i] for i in range(4)]
            b2c = [cols[:, 8:9], cols[:, 9:10]]
            zcol = cols[:, 10:11]
            epscol = cols[:, 11:12]
            sumb1c = cols[:, 12:13]  # sum(ln1_b)/D

            ones = wp.tile([1, 128], BF, name="ones", tag="ones")
            nc.vector.memset(ones[:], 1.0)

            att = [ps_at.tile([128, H, 33], F32, tag=f"att{qt}",
                              name=f"att{qt}") for qt in range(2)]

            # PE clock warm-up: dummy matmuls into the (not yet used) att
            # psum banks keep the PE busy from t=0 so its p-state ramps to
            # full clock before the real pipeline starts. att is memset
            # afterwards, before the attnV accumulation begins.
            def warm(n, qt=0):
                dst = att[qt][:].rearrange("p h e -> p (h e)")
                wrhs = ones[0:1, 0:1].broadcast_to((1, 264))
                for _ in range(n):
                    nc.tensor.matmul(dst, ones[:], wrhs,
                                     start=True, stop=True,
                                     skip_group_check=True)

            # V with per-head all-ones denominator column (col 32 of 33)
            v_sb = wp.tile([128, NCH, H, 33], BF, name="v_sb", tag="v_sb")
            nc.vector.memset(v_sb[:, :, :, 32:33], 1.0)

            # ---------- Q projection ----------
            warm(8, 0)
            QT = []
            for oc in range(2):
                ps = ps_ms.tile([128, 512], F32, tag="pms", name="psq")
                for ic in range(2):
                    nc.tensor.matmul(
                        ps[:, 0:QS], wq[ic][:, 128 * oc:128 * oc + 128],
                        hTs[ic][:], start=(ic == 0), stop=(ic == 1))
                t = wp.tile([128, QS], BF, name=f"QT{oc}", tag=f"QT{oc}")
                nc.vector.tensor_scalar_add(t[:], ps[:, 0:QS], bq[oc])
                QT.append(t)

            KT = [wp.tile([128, N], BF, name=f"KT{i}", tag=f"KT{i}")
                  for i in range(2)]
            pt_tiles = {}

            def emit_kproj(c0, c1):
                w = c1 - c0
                for oc in range(2):
                    ps = ps_ms.tile([128, 512], F32, tag="pms", name="psk")
                    for ic in range(2):
                        rhs = (hT0[ic][:, c0:c1] if c1 <= 256
                               else hT[ic][:, c0:c1])
                        nc.tensor.matmul(
                            ps[:, 0:w], wk[ic][:, 128 * oc:128 * oc + 128],
                            rhs, start=(ic == 0), stop=(ic == 1))
                    nc.vector.tensor_scalar_add(
                        KT[oc][:, c0:c1], ps[:, 0:w], bk[oc])

            def emit_vproj(c):
                ps = ps_ms.tile([128, 512], F32, tag="pms", name="psv")
                for ic in range(2):
                    lhs = (hT0[ic][:, 128 * c:128 * c + 128] if c < 2
                           else hT[ic][:, 128 * c:128 * c + 128])
                    nc.tensor.matmul(
                        ps[:, 0:D], lhs,
                        wv[ic][:], start=(ic == 0), stop=(ic == 1))
                nc.vector.tensor_copy(
                    v_sb[:, c, :, 0:32],
                    ps[:, 0:D].rearrange("p (h e) -> p h e", h=H))

            def emit_chunk(c):
                pt = ptp.tile([128, 2, 4, QS], BF, tag="pt", name=f"pt{c}")
                pt_tiles[c] = pt
                bt = bias_tiles.pop(c)
                for g in range(2):
                    ps = ps_sc.tile([128, 4, QS], F32, tag="sc",
                                    name=f"sc{c}_{g}")
                    for s in range(2):
                        rhs = bt[:, 4 * g + 2 * s:4 * g + 2 * s + 2, :]
                        rhs = rhs.rearrange("p a b -> p (a b)")
                        rhs = rhs.unsqueeze(1).broadcast_to((128, 2, 512))
                        nc.tensor.matmul(
                            ps[:, 2 * s:2 * s + 2, :], id8[:], rhs,
                            start=True, stop=False, skip_group_check=True,
                            perf_mode=mybir.MatmulPerfMode.DoubleRow)
                    for hh in range(4):
                        h = ORDER[4 * g + hh]
                        bp = 32 * (h % 4)
                        nc.tensor.matmul(
                            ps[:, hh, :],
                            KT[h // 4][bp:bp + 32, 128 * c:128 * c + 128],
                            QT[h // 4][bp:bp + 32, :],
                            start=False, stop=True, tile_position=(bp, 0),
                            skip_group_check=True)
                    nc.scalar.activation(pt[:, g], ps[:], AF.Exp, bias=zcol)

            def emit_attnv(c):
                pt = pt_tiles.pop(c)
                for qt in range(2):
                    for g in range(2):
                        for hh in range(4):
                            h = ORDER[4 * g + hh]
                            nc.tensor.matmul(
                                att[qt][:, h, :],
                                pt[:, g, hh, 128 * qt:128 * qt + 128],
                                v_sb[:, c, h, :],
                                start=False, stop=(c == NCH - 1),
                                skip_group_check=True)

            def emit_c(c):
                if c + 4 < NCH:
                    emit_bias_dma(c + 4)
                emit_vproj(c)
                emit_chunk(c)
                if c >= 2:
                    emit_attnv(c - 2)

            # ---------- main pipeline ----------
            warm(3, 1)
            for qt in range(2):
                nc.vector.memset(att[qt][:], 0.0)
            emit_kproj(0, 256)
            emit_c(0)
            emit_kproj(256, 512)
            emit_c(1)
            emit_kproj(512, 1024)
            emit_c(2)
            emit_c(3)
            emit_kproj(1024, 1536)
            emit_c(4)
            emit_c(5)
            emit_kproj(1536, 2048)

            # deferred (epilogue-only) loads
            epk = [load(wp, d_epack, [128, 4 * D], f"epk{i}", BF, 128 * i)
                   for i in range(2)]
            hres = [epk[i][:, 0:D] for i in range(2)]
            wo = [epk[i][:, D:2 * D] for i in range(2)]
            w1 = [epk[i][:, 2 * D:4 * D] for i in range(2)]
            w2l = load(wp, d_w2lng, [128, 8 * D], "w2l", BF)
            w2 = [w2l[:, D * i:D * (i + 1)] for i in range(4)]
            lng = w2l[:, 4 * D:8 * D]
            idbo = load(wp, d_idbo, [128, 384], "idbo", BF)
            idb = idbo[:, 0:128]
            bo = idbo[0:1, 128:384]

            for c in range(6, NCH):
                emit_c(c)
            emit_attnv(NCH - 2)
            emit_attnv(NCH - 1)

            # ---------- epilogue (stage-interleaved across qt) ----------
            o_nat = [wp.tile([128, D], BF, name=f"onat{qt}", tag=f"onat{qt}")
                     for qt in range(2)]
            OT = [wp.tile([128, D], BF, name=f"OT{fc}", tag=f"OT{fc}")
                  for fc in range(2)]
            fT = [wp.tile([128, D], BF, name=f"fT{fc}", tag=f"fT{fc}")
                  for fc in range(2)]
            g1T = [wp.tile([128, QS], BF, name=f"g1T{oc}", tag=f"g1T{oc}")
                   for oc in range(4)]
            y2T = [wp.tile([128, QS], BF, name=f"y2T{oc}", tag=f"y2T{oc}")
                   for oc in range(2)]
            h1 = [wp.tile([128, D], F32, name=f"h1_{qt}", tag=f"h1_{qt}")
                  for qt in range(2)]
            xin = [sm.tile([128, D], F32, tag=f"xin{qt}", name=f"xin{qt}")
                   for qt in range(2)]
            fln = [sm.tile([128, D], BF, tag=f"fln{qt}", name=f"fln{qt}")
                   for qt in range(2)]
            out_sb = [wp.tile([128, D], F32, name=f"osb{qt}", tag=f"osb{qt}")
                      for qt in range(2)]

            # normalize attention output (single broadcast multiply);
            # both reciprocals first so qt1 doesn't queue behind qt0's mult
            rdts = []
            for qt in range(2):
                rdt = sm.tile([128, H, 1], F32, tag=f"rd{qt}",
                              name=f"rd{qt}")
                nc.vector.reciprocal(rdt[:], att[qt][:, :, 32:33])
                rdts.append(rdt)
            for qt in range(2):
                nc.vector.tensor_mul(
                    o_nat[qt][:].rearrange("p (h e) -> p h e", h=H),
                    att[qt][:, :, 0:32],
                    rdts[qt][:].broadcast_to((128, H, 32)))
            def tr_ps(qt, name):
                if qt == 0:
                    return ps_ms.tile([128, 512], BF, tag="pms",
                                      name=name)[:, 0:128]
                return ps_sc.tile([128, 8, 128], BF, tag="sc",
                                  name=name)[:, 0, :]

            # transpose o_nat -> OT
            for qt in range(2):
                for fc in range(2):
                    tp = tr_ps(qt, "trp")
                    nc.tensor.transpose(
                        tp, o_nat[qt][:, 128 * fc:128 * fc + 128],
                        idb[:])
                    nc.vector.tensor_copy(
                        OT[fc][:, 128 * qt:128 * qt + 128], tp)
            # out-projection + residual
            for qt in range(2):
                if qt == 0:
                    aps = ps_ms.tile([128, 512], F32, tag="pms", name="apo")
                else:
                    aps = ps_sc.tile([128, 4, QS], F32, tag="sc",
                                     name="apo")[:, 0:2, :].rearrange(
                                         "p a b -> p (a b)")
                for ic in range(2):
                    nc.tensor.matmul(
                        aps[:, 0:D], OT[ic][:, 128 * qt:128 * qt + 128],
                        wo[ic][:], start=(ic == 0), stop=False)
                nc.tensor.matmul(aps[:, 0:D], ones[:], bo[:],
                                 start=False, stop=True)
                nc.vector.tensor_add(xin[qt][:], aps[:, 0:D], hres[qt][:])

            def ln_stats(x_ap, tagp, eng):
                """Emit LN stats for one tile; returns (x, nm, st)."""
                ss = sm.tile([128, 1], F32, tag=f"{tagp}ss", name="ss")
                nc.vector.reduce_sum(ss[:], x_ap, axis=AX.X)
                nm = sm.tile([128, 1], F32, tag=f"{tagp}nm", name="nm")
                eng.tensor_scalar_mul(nm[:], ss[:], -1.0 / D)
                scr = sm.tile([128, D], F32, tag=f"{tagp}scr", name="scr")
                vs = sm.tile([128, 1], F32, tag=f"{tagp}vs", name="vs")
                nc.scalar.activation(scr[:], x_ap, AF.Square, bias=nm[:],
                                     accum_out=vs[:])
                st = sm.tile([128, 1], F32, tag=f"{tagp}st", name="st")
                nc.scalar.activation(st[:], vs[:], AF.Sqrt, bias=epscol,
                                     scale=1.0 / D)
                return x_ap, nm, st

            def ln_apply(x_ap, nm, st, g_ap, b_ap, out_ap, tagp, eng):
                r0 = sm.tile([128, 1], F32, tag=f"{tagp}r0", name="r0")
                nc.vector.reciprocal(r0[:], st[:])
                if g_ap is None:  # gamma/beta folded into next matmul
                    eng.tensor_scalar(out_ap, x_ap, nm[:], r0[:],
                                      op0=ALU.add, op1=ALU.mult)
                    return None
                t1 = sm.tile([128, D], F32, tag=f"{tagp}t1", name="t1")
                eng.scalar_tensor_tensor(t1[:], x_ap, nm[:], g_ap,
                                         ALU.add, ALU.mult)
                eng.scalar_tensor_tensor(out_ap, t1[:], r0[:], b_ap,
                                         ALU.mult, ALU.add)
                return t1, r0

            LNE = [nc.vector, nc.vector]

            # LN1 both qt (Sqrt ops batched; qt1 runs on gpsimd)
            st1 = []
            for qt in range(2):
                st1.append(ln_stats(xin[qt][:], f"l1q{qt}", LNE[qt]))
            t1r0 = []
            for qt in range(2):
                x_ap, nm, st = st1[qt]
                t1r0.append(ln_apply(x_ap, nm, st, lng[:, 0:D],
                                     lng[:, D:2 * D], h1[qt][:],
                                     f"l1q{qt}", LNE[qt]))
            # LN2: mean from t1 (sum(h1) = r0*sum(t1) + sum(b1ln)) so the
            # reduce runs off the h1 critical path; variance from h1.
            st2 = []
            for qt in range(2):
                t1, r0 = t1r0[qt]
                sst1 = sm.tile([128, 1], F32, tag=f"l2s{qt}", name="sst1")
                nc.vector.reduce_sum(sst1[:], t1[:], axis=AX.X)
                tmr = sm.tile([128, 1], F32, tag=f"l2t{qt}", name="tmr")
                nc.vector.tensor_mul(tmr[:], sst1[:], r0[:])
                nm2 = sm.tile([128, 1], F32, tag=f"l2n{qt}", name="nm2")
                nc.vector.tensor_scalar(
                    nm2[:], tmr[:], -1.0 / D, sumb1c,
                    op0=ALU.mult, op1=ALU.subtract)
                scr = sm.tile([128, D], F32, tag=f"l2c{qt}", name="scr2")
                vs = sm.tile([128, 1], F32, tag=f"l2v{qt}", name="vs2")
                nc.scalar.activation(scr[:], h1[qt][:], AF.Square,
                                     bias=nm2[:], accum_out=vs[:])
                st = sm.tile([128, 1], F32, tag=f"l2w{qt}", name="st2")
                nc.scalar.activation(st[:], vs[:], AF.Sqrt, bias=epscol,
                                     scale=1.0 / D)
                st2.append((nm2, st))
            for qt in range(2):
                nm2, st = st2[qt]
                ln_apply(h1[qt][:], nm2, st, None, None,
                         fln[qt][:], f"l2q{qt}", LNE[qt])
            # transpose fln -> fT
            for qt in range(2):
                for fc in range(2):
                    tp2 = tr_ps(qt, "trf")
                    nc.tensor.transpose(
                        tp2, fln[qt][:, 128 * fc:128 * fc + 128],
                        idb[:])
                    nc.vector.tensor_copy(
                        fT[fc][:, 128 * qt:128 * qt + 128], tp2)
            # FFN1 + gelu (gelus batched); psum from the now-idle score pool
            for qt in range(2):
                for oc in range(4):
                    fs = ps_sc.tile([128, 4, QS], F32, tag="sc", name="ps1")
                    fp = fs[:, 0, :]
                    for ic in range(2):
                        nc.tensor.matmul(
                            fp[:, 0:128],
                            w1[ic][:, 128 * oc:128 * oc + 128],
                            fT[ic][:, 128 * qt:128 * qt + 128],
                            start=(ic == 0), stop=(ic == 1))
                    nc.scalar.activation(
                        g1T[oc][:, 128 * qt:128 * qt + 128], fp[:, 0:128],
                        AF.Gelu, bias=b1c[oc])
            # FFN2 + transpose back + residual + store (per qt)
            for qt in range(2):
                for oc in range(2):
                    fs = ps_sc.tile([128, 4, QS], F32, tag="sc", name="ps2")
                    fp = fs[:, 0, :]
                    for ic in range(4):
                        nc.tensor.matmul(
                            fp[:, 0:128],
                            w2[ic][:, 128 * oc:128 * oc + 128],
                            g1T[ic][:, 128 * qt:128 * qt + 128],
                            start=(ic == 0), stop=(ic == 3))
                    nc.scalar.activation(
                        y2T[oc][:, 128 * qt:128 * qt + 128], fp[:, 0:128],
                        AF.Identity, bias=b2c[oc])
                for fc in range(2):
                    tp3 = tr_ps(qt, "trn")
                    nc.tensor.transpose(
                        tp3, y2T[fc][:, 128 * qt:128 * qt + 128],
                        idb[:])
                    nc.vector.tensor_add(
                        out_sb[qt][:, 128 * fc:128 * fc + 128],
                        h1[qt][:, 128 * fc:128 * fc + 128], tp3)
                nc.sync.dma_start(d_out.ap()[128 * qt:128 * qt + 128, :],
                                  out_sb[qt][:])

    nc.compile()
    return nc


_CACHE = {}
USE_FR = True


def _get_nc(use_fr=True):
    if use_fr not in _CACHE:
        _CACHE[use_fr] = build_kernel()
    return _CACHE[use_fr]


def kernel(**inputs):
    import ml_dtypes

    bf = ml_dtypes.bfloat16
    f8 = ml_dtypes.float8_e4m3fn

    h = np.asarray(inputs["h"], np.float32)
    edge_attr = np.asarray(inputs["edge_attr"], np.float32)
    edge_index = np.asarray(inputs["edge_index"])
    Wq, bq = np.asarray(inputs["Wq"], np.float32), np.asarray(inputs["bq"], np.float32)
    Wk, bk = np.asarray(inputs["Wk"], np.float32), np.asarray(inputs["bk"], np.float32)
    Wv, bv = np.asarray(inputs["Wv"], np.float32), np.asarray(inputs["bv"], np.float32)
    Wo, bo = np.asarray(inputs["Wo"], np.float32), np.asarray(inputs["bo"], np.float32)
    We, be = np.asarray(inputs["We"], np.float32), np.asarray(inputs["be"], np.float32)
    ln1_g, ln1_b = np.asarray(inputs["ln1_g"], np.float32), np.asarray(inputs["ln1_b"], np.float32)
    fln_g, fln_b = np.asarray(inputs["fln_g"], np.float32), np.asarray(inputs["fln_b"], np.float32)
    W1, b1 = np.asarray(inputs["W1"], np.float32), np.asarray(inputs["b1"], np.float32)
    W2, b2 = np.asarray(inputs["W2"], np.float32), np.asarray(inputs["b2"], np.float32)

    scale = 1.0 / np.sqrt(np.float32(DK))
    eb = edge_attr @ We + be  # (E, H)
    hT = np.ascontiguousarray(h.T)

    cols = np.zeros((128, 16), np.float32)
    cols[:, 0] = bq[0:128] * scale
    cols[:, 1] = bq[128:256] * scale
    cols[:, 2] = bk[0:128]
    cols[:, 3] = bk[128:256]
    b1_eff = b1 + fln_b @ W1
    for i in range(4):
        cols[:, 4 + i] = b1_eff[128 * i:128 * i + 128]
    cols[:, 8] = b2[0:128]
    cols[:, 9] = b2[128:256]
    cols[:, 11] = EPS
    cols[:, 12] = float(ln1_b.sum()) / D

    bo_eff = (bv @ Wo + bo).reshape(1, D)
    lng = np.concatenate(
        [np.tile(x.reshape(1, D), (128, 1))
         for x in [ln1_g, ln1_b, fln_g, fln_b]], axis=1)

    w1_eff = fln_g.reshape(D, 1) * W1
    idbo = np.zeros((128, 384), np.float32)
    idbo[:, 0:128] = np.eye(128)
    idbo[0, 128:384] = bo_eff[0]
    w2lng = np.concatenate(
        [W2[0:128], W2[128:256], W2[256:384], W2[384:512], lng], axis=1)
    common = {
        "hT": hT.astype(bf),
        "w2lng": w2lng.astype(bf),
        "idbo": idbo.astype(bf),
    }
    id8dr = np.stack(
        [np.eye(128, dtype=np.float32),
         np.zeros((128, 128), np.float32)], axis=1).reshape(128, 256)
    wq_s = Wq * scale

    src = edge_index[0].astype(np.int64)
    dst = edge_index[1].astype(np.int64)
    in_maps = []
    for c in range(N_CORES):
        r0 = c * QS
        m = dict(common)
        qp = np.concatenate(
            [hT[:, 0:QS], hT[:, r0:r0 + QS], wq_s, Wk, Wv],
            axis=1).astype(bf)
        colsb = cols.astype(np.float32).view(bf).reshape(128, 32)
        m["qpack"] = np.ascontiguousarray(np.concatenate(
            [qp, np.concatenate([colsb, np.zeros_like(colsb)], axis=0)],
            axis=1))
        hres_c = h[r0:r0 + QS]
        m["epack"] = np.ascontiguousarray(np.concatenate(
            [np.concatenate([hres_c[128 * b:128 * b + 128],
                             Wo[128 * b:128 * b + 128],
                             w1_eff[128 * b:128 * b + 128]], axis=1)
             for b in range(2)], axis=0)).astype(bf)
        sel = (src >= r0) & (src < r0 + QS)
        slab = np.zeros((N, H, QS), np.float32)
        slab[dst[sel], :, src[sel] - r0] = eb[sel]
        slab = slab[:, ORDER, :]  # head slot order
        slab = slab.reshape(NCH, 128, H * QS)
        m["bias8"] = np.ascontiguousarray(slab).astype(f8).reshape(
            NCH, 128, H, QS)
        m["b0pack"] = np.ascontiguousarray(np.concatenate(
            [id8dr, slab[0]], axis=1)).astype(f8)
        in_maps.append(m)

    nc = _get_nc(USE_FR)
    res = run_bass_kernel_spmd(nc, in_maps, core_ids=list(range(N_CORES)))
    out = np.concatenate([res.results[cc]["out"] for cc in range(N_CORES)],
                         axis=0)
    return out.astype(np.float32)


# revision 48
# speedup vs baseline: 1.0184x; 1.0063x over previous
"""GraphTransformerLayer on 8 TRN2 NeuronCores (Bass/Tile).

Sharding: query/node dim N=2048 split into 8 shards of 256 rows; each core
holds replicated K/V for all 2048 keys plus its 256-query shard.

Design (v3):
- All matmuls bf16 (fp32 runs at 1/4 PE rate); psum accumulation f32.
- Edge bias: dense per-core fp8 slab [16][128 keys, 8 heads, 256 queries],
  added into the score PSUM by seeding each accumulation group with an
  fp8 identity-matmul (start=True) before the K^T Q score matmuls
  (start=False, stop=True) accumulate on top.
- Heads are processed in the order [0,4,1,5,2,6,3,7] so that the two heads
  sharing one 2KB psum zero-region use the same PE quadrant row offset —
  mixing different sub-128 partition offsets in one zero region hangs the
  PE on hardware.
- Scores chunk-major: per key-chunk c (128 keys) and half g (4 heads), one
  [128, 4x256] psum tile -> one Exp activation psum->SBUF bf16 (pt).
  Unnormalized softmax; denominator comes from an all-ones column per head
  appended to V (attn @ [V|1]).
- attnV accumulates into memset-seeded persistent [128, 8, 33] psum tiles
  per query half (start=False throughout).
- V projection is spread across the chunk loop as PE filler; epilogue is
  stage-interleaved across the two query halves with activation-table
  switches batched (Exp -> Sqrt -> Gelu).
"""

import sys

sys.path.insert(0, "/opt/trn_rl_repo")

import numpy as np

import concourse.bacc as bacc
import concourse.mybir as mybir
import concourse.tile as tile
from concourse.bass_utils import run_bass_kernel_spmd

N_CORES = 8
N = 2048
D = 256
H = 8
DK = 32
QS = 256
H2 = 512
EPS = 1e-5
NCH = 16  # key chunks of 128

F32 = mybir.dt.float32
BF = mybir.dt.bfloat16
F8 = mybir.dt.float8e4

AF = mybir.ActivationFunctionType
ALU = mybir.AluOpType
AX = mybir.AxisListType

# Head processing order: pairs sharing a PE quadrant row offset (bp) share
# a psum zero region.
ORDER = [0, 4, 1, 5, 2, 6, 3, 7]


def build_kernel():
    nc = bacc.Bacc("TRN2", target_bir_lowering=False, debug=False,
                   num_devices=N_CORES)

    d_hT = nc.dram_tensor("hT", [D, N], BF, kind="ExternalInput")
    # qpack rows=feature dims; cols = [hT[:,0:256] | hTs | wq | wk | wv |
    # cols-as-bf16-bytes (24)]
    d_qpack = nc.dram_tensor("qpack", [D, 5 * D + 32], BF,
                             kind="ExternalInput")
    # epack row-blocks of 128; cols = [hres | wo | w1]
    d_epack = nc.dram_tensor("epack", [D, 4 * D], BF, kind="ExternalInput")
    # w2lng cols = [w2 blocks 0..3 | lng g1,b1,g2,b2]
    d_w2lng = nc.dram_tensor("w2lng", [128, 8 * D], BF, kind="ExternalInput")
    # idbo: [:,0:128]=identity bf16; row0 cols 128:384 = bo_eff
    d_idbo = nc.dram_tensor("idbo", [128, 384], BF, kind="ExternalInput")
    d_bias = nc.dram_tensor("bias8", [NCH, 128, H, QS], F8,
                            kind="ExternalInput")
    # b0pack: [:,0:256]=DoubleRow identity (fp8), [:,256:2304]=bias chunk 0
    d_b0 = nc.dram_tensor("b0pack", [128, 2304], F8, kind="ExternalInput")
    d_out = nc.dram_tensor("out", [QS, D], F32, kind="ExternalOutput")

    with tile.TileContext(nc) as tc:
        import contextlib

        with contextlib.ExitStack() as ctx:
            wp = ctx.enter_context(tc.tile_pool(name="w", bufs=1))
            bpool = ctx.enter_context(tc.tile_pool(name="bias", bufs=6))
            ptp = ctx.enter_context(tc.tile_pool(name="pt", bufs=6))
            sm = ctx.enter_context(tc.tile_pool(name="sm", bufs=2))
            ps_sc = ctx.enter_context(
                tc.tile_pool(name="psc", bufs=2, space="PSUM"))
            ps_at = ctx.enter_context(
                tc.tile_pool(name="pat", bufs=1, space="PSUM"))
            ps_ms = ctx.enter_context(
                tc.tile_pool(name="pms", bufs=2, space="PSUM"))

            def load(pool, dram, shape, name, dt, r0=0, c0=0):
                t = pool.tile(shape, dt, name=name, tag=name)
                nc.sync.dma_start(
                    t[:], dram.ap()[r0:r0 + shape[0], c0:c0 + shape[1]])
                return t

            # ---- critical-path loads first ----
            # hT loaded in column pieces so K-proj can start early
            hT = [wp.tile([128, N], BF, name=f"hT{i}", tag=f"hT{i}")
                  for i in range(2)]

            def load_hT_piece(c0, c1):
                for i in range(2):
                    nc.sync.dma_start(
                        hT[i][:, c0:c1],
                        d_hT.ap()[128 * i:128 * i + 128, c0:c1])

            bias_tiles = {}

            def emit_bias_dma(c):
                t = bpool.tile([128, H, QS], F8, tag="bias", name=f"bias{c}")
                nc.sync.dma_start(t[:], d_bias.ap()[c])
                bias_tiles[c] = t

            qpk = [load(wp, d_qpack, [128, 5 * D + 32], f"qpk{i}", BF,
                        128 * i) for i in range(2)]
            hT0 = [qpk[i][:, 0:D] for i in range(2)]
            hTs = [qpk[i][:, D:2 * D] for i in range(2)]
            wq = [qpk[i][:, 2 * D:3 * D] for i in range(2)]
            wk = [qpk[i][:, 3 * D:4 * D] for i in range(2)]
            wv = [qpk[i][:, 4 * D:5 * D] for i in range(2)]
            cols = qpk[0][:, 5 * D:5 * D + 32].bitcast(F32)
            b0t = wp.tile([128, 2304], F8, name="b0t", tag="b0t")
            nc.sync.dma_start(b0t[:], d_b0.ap()[:, :])
            id8 = b0t[:, 0:256].rearrange("p (a b) -> p a b", a=2)
            bias_tiles[0] = b0t[:, 256:2304].rearrange(
                "p (h q) -> p h q", h=H)
            load_hT_piece(256, 1024)
            emit_bias_dma(1)
            load_hT_piece(1024, 2048)
            emit_bias_dma(2)
            emit_bias_dma(3)

            bq = [cols[:, 0:1], cols[:, 1:2]]
            bk = [cols[:, 2:3], cols[:, 3:4]]
            b1c = [cols[:, 4 + i:5 + i] for i in range(4)]
            b2c = [cols[:, 8:9], cols[:, 9:10]]
            zcol = cols[:, 10:11]
            epscol = cols[:, 11:12]
            sumb1c = cols[:, 12:13]  # sum(ln1_b)/D

            ones = wp.tile([1, 128], BF, name="ones", tag="ones")
            nc.vector.memset(ones[:], 1.0)

            att = [ps_at.tile([128, H, 33], F32, tag=f"att{qt}",
                              name=f"att{qt}") for qt in range(2)]

            # PE clock warm-up: dummy matmuls into the (not yet used) att
            # psum banks keep the PE busy from t=0 so its p-state ramps to
            # full clock before the real pipeline starts. att is memset
            # afterwards, before the attnV accumulation begins.
            def warm(n, qt=0):
                dst = att[qt][:].rearrange("p h e -> p (h e)")
                wrhs = ones[0:1, 0:1].broadcast_to((1, 264))
                for _ in range(n):
                    nc.tensor.matmul(dst, ones[:], wrhs,
                                     start=True, stop=True,
                                     skip_group_check=True)

            # V with per-head all-ones denominator column (col 32 of 33)
            v_sb = wp.tile([128, NCH, H, 33], BF, name="v_sb", tag="v_sb")
            nc.vector.memset(v_sb[:, :, :, 32:33], 1.0)

            # ---------- Q projection ----------
            warm(8, 0)
            QT = []
            for oc in range(2):
                ps = ps_ms.tile([128, 512], F32, tag="pms", name="psq")
                for ic in range(2):
                    nc.tensor.matmul(
                        ps[:, 0:QS], wq[ic][:, 128 * oc:128 * oc + 128],
                        hTs[ic][:], start=(ic == 0), stop=(ic == 1))
                t = wp.tile([128, QS], BF, name=f"QT{oc}", tag=f"QT{oc}")
                nc.vector.tensor_scalar_add(t[:], ps[:, 0:QS], bq[oc])
                QT.append(t)

            KT = [wp.tile([128, N], BF, name=f"KT{i}", tag=f"KT{i}")
                  for i in range(2)]
            pt_tiles = {}

            def emit_kproj(c0, c1):
                w = c1 - c0
                for oc in range(2):
                    ps = ps_ms.tile([128, 512], F32, tag="pms", name="psk")
                    for ic in range(2):
                        rhs = (hT0[ic][:, c0:c1] if c1 <= 256
                               else hT[ic][:, c0:c1])
                        nc.tensor.matmul(
                            ps[:, 0:w], wk[ic][:, 128 * oc:128 * oc + 128],
                            rhs, start=(ic == 0), stop=(ic == 1))
                    nc.vector.tensor_scalar_add(
                        KT[oc][:, c0:c1], ps[:, 0:w], bk[oc])

            def emit_vproj(c):
                ps = ps_ms.tile([128, 512], F32, tag="pms", name="psv")
                for ic in range(2):
                    lhs = (hT0[ic][:, 128 * c:128 * c + 128] if c < 2
                           else hT[ic][:, 128 * c:128 * c + 128])
                    nc.tensor.matmul(
                        ps[:, 0:D], lhs,
                        wv[ic][:], start=(ic == 0), stop=(ic == 1))
                nc.vector.tensor_copy(
                    v_sb[:, c, :, 0:32],
                    ps[:, 0:D].rearrange("p (h e) -> p h e", h=H))

            def emit_chunk(c):
                pt = ptp.tile([128, 2, 4, QS], BF, tag="pt", name=f"pt{c}")
                pt_tiles[c] = pt
                bt = bias_tiles.pop(c)
                for g in range(2):
                    ps = ps_sc.tile([128, 4, QS], F32, tag="sc",
                                    name=f"sc{c}_{g}")
                    for s in range(2):
                        rhs = bt[:, 4 * g + 2 * s:4 * g + 2 * s + 2, :]
                        rhs = rhs.rearrange("p a b -> p (a b)")
                        rhs = rhs.unsqueeze(1).broadcast_to((128, 2, 512))
                        nc.tensor.matmul(
                            ps[:, 2 * s:2 * s + 2, :], id8[:], rhs,
                            start=True, stop=False, skip_group_check=True,
                            perf_mode=mybir.MatmulPerfMode.DoubleRow)
                    for hh in range(4):
                        h = ORDER[4 * g + hh]
                        bp = 32 * (h % 4)
                        nc.tensor.matmul(
                            ps[:, hh, :],
                            KT[h // 4][bp:bp + 32, 128 * c:128 * c + 128],
                            QT[h // 4][bp:bp + 32, :],
                            start=False, stop=True, tile_position=(bp, 0),
                            skip_group_check=True)
                    nc.scalar.activation(pt[:, g], ps[:], AF.Exp, bias=zcol)

            def emit_attnv(c):
                pt = pt_tiles.pop(c)
                for qt in range(2):
                    for g in range(2):
                        for hh in range(4):
                            h = ORDER[4 * g + hh]
                            nc.tensor.matmul(
                                att[qt][:, h, :],
                                pt[:, g, hh, 128 * qt:128 * qt + 128],
                                v_sb[:, c, h, :],
                                start=False, stop=(c == NCH - 1),
                                skip_group_check=True)

            def emit_c(c):
                if c + 4 < NCH:
                    emit_bias_dma(c + 4)
                emit_vproj(c)
                emit_chunk(c)
                if c >= 2:
                    emit_attnv(c - 2)

            # ---------- main pipeline ----------
            warm(3, 1)
            for qt in range(2):
                nc.vector.memset(att[qt][:], 0.0)
            emit_kproj(0, 256)
            emit_c(0)
            emit_kproj(256, 512)
            emit_c(1)
            emit_kproj(512, 1024)
            emit_c(2)
            emit_c(3)
            emit_kproj(1024, 1536)
            emit_c(4)
            emit_c(5)
            emit_kproj(1536, 2048)

            # deferred (epilogue-only) loads
            epk = [load(wp, d_epack, [128, 4 * D], f"epk{i}", BF, 128 * i)
                   for i in range(2)]
            hres = [epk[i][:, 0:D] for i in range(2)]
            wo = [epk[i][:, D:2 * D] for i in range(2)]
            w1 = [epk[i][:, 2 * D:4 * D] for i in range(2)]
            w2l = load(wp, d_w2lng, [128, 8 * D], "w2l", BF)
            w2 = [w2l[:, D * i:D * (i + 1)] for i in range(4)]
            lng = w2l[:, 4 * D:8 * D]
            idbo = load(wp, d_idbo, [128, 384], "idbo", BF)
            idb = idbo[:, 0:128]
            bo = idbo[0:1, 128:384]

            for c in range(6, NCH):
                emit_c(c)
            emit_attnv(NCH - 2)
            emit_attnv(NCH - 1)

            # ---------- epilogue (stage-interleaved across qt) ----------
            o_nat = [wp.tile([128, D], BF, name=f"onat{qt}", tag=f"onat{qt}")
                     for qt in range(2)]
            OT = [wp.tile([128, D], BF, name=f"OT{fc}", tag=f"OT{fc}")
                  for fc in range(2)]
            fT = [wp.tile([128, D], BF, name=f"fT{fc}", tag=f"fT{fc}")
                  for fc in range(2)]
            g1T = [wp.tile([128, QS], BF, name=f"g1T{oc}", tag=f"g1T{oc}")
                   for oc in range(4)]
            y2T = [wp.tile([128, QS], BF, name=f"y2T{oc}", tag=f"y2T{oc}")
                   for oc in range(2)]
            h1 = [wp.tile([128, D], F32, name=f"h1_{qt}", tag=f"h1_{qt}")
                  for qt in range(2)]
            xin = [sm.tile([128, D], F32, tag=f"xin{qt}", name=f"xin{qt}")
                   for qt in range(2)]
            fln = [sm.tile([128, D], BF, tag=f"fln{qt}", name=f"fln{qt}")
                   for qt in range(2)]
            out_sb = [wp.tile([128, D], F32, name=f"osb{qt}", tag=f"osb{qt}")
                      for qt in range(2)]

            # normalize attention output (single broadcast multiply);
            # both reciprocals first so qt1 doesn't queue behind qt0's mult
            rdts = []
            for qt in range(2):
                rdt = sm.tile([128, H, 1], F32, tag=f"rd{qt}",
                              name=f"rd{qt}")
                nc.vector.reciprocal(rdt[:], att[qt][:, :, 32:33])
                rdts.append(rdt)
            for qt in range(2):
                nc.vector.tensor_mul(
                    o_nat[qt][:].rearrange("p (h e) -> p h e", h=H),
                    att[qt][:, :, 0:32],
                    rdts[qt][:].broadcast_to((128, H, 32)))
            def tr_ps(qt, name):
                if qt == 0:
                    return ps_ms.tile([128, 512], BF, tag="pms",
                                      name=name)[:, 0:128]
                return ps_sc.tile([128, 8, 128], BF, tag="sc",
                                  name=name)[:, 0, :]

            # transpose o_nat -> OT
            for qt in range(2):
                for fc in range(2):
                    tp = tr_ps(qt, "trp")
                    nc.tensor.transpose(
                        tp, o_nat[qt][:, 128 * fc:128 * fc + 128],
                        idb[:])
                    nc.vector.tensor_copy(
                        OT[fc][:, 128 * qt:128 * qt + 128], tp)
            # out-projection + residual
            for qt in range(2):
                if qt == 0:
                    aps = ps_ms.tile([128, 512], F32, tag="pms", name="apo")
                else:
                    aps = ps_sc.tile([128, 4, QS], F32, tag="sc",
                                     name="apo")[:, 0:2, :].rearrange(
                                         "p a b -> p (a b)")
                for ic in range(2):
                    nc.tensor.matmul(
                        aps[:, 0:D], OT[ic][:, 128 * qt:128 * qt + 128],
                        wo[ic][:], start=(ic == 0), stop=False)
                nc.tensor.matmul(aps[:, 0:D], ones[:], bo[:],
                                 start=False, stop=True)
                nc.vector.tensor_add(xin[qt][:], aps[:, 0:D], hres[qt][:])

            def ln_stats(x_ap, tagp, eng):
                """Emit LN stats for one tile; returns (x, nm, st)."""
                ss = sm.tile([128, 1], F32, tag=f"{tagp}ss", name="ss")
                nc.vector.reduce_sum(ss[:], x_ap, axis=AX.X)
                nm = sm.tile([128, 1], F32, tag=f"{tagp}nm", name="nm")
                eng.tensor_scalar_mul(nm[:], ss[:], -1.0 / D)
                scr = sm.tile([128, D], F32, tag=f"{tagp}scr", name="scr")
                vs = sm.tile([128, 1], F32, tag=f"{tagp}vs", name="vs")
                nc.scalar.activation(scr[:], x_ap, AF.Square, bias=nm[:],
                                     accum_out=vs[:])
                st = sm.tile([128, 1], F32, tag=f"{tagp}st", name="st")
                nc.scalar.activation(st[:], vs[:], AF.Sqrt, bias=epscol,
                                     scale=1.0 / D)
                return x_ap, nm, st

            def ln_apply(x_ap, nm, st, g_ap, b_ap, out_ap, tagp, eng):
                r0 = sm.tile([128, 1], F32, tag=f"{tagp}r0", name="r0")
                nc.vector.reciprocal(r0[:], st[:])
                if g_ap is None:  # gamma/beta folded into next matmul
                    eng.tensor_scalar(out_ap, x_ap, nm[:], r0[:],
                                      op0=ALU.add, op1=ALU.mult)
                    return None
                t1 = sm.tile([128, D], F32, tag=f"{tagp}t1", name="t1")
                eng.scalar_tensor_tensor(t1[:], x_ap, nm[:], g_ap,
                                         ALU.add, ALU.mult)
                eng.scalar_tensor_tensor(out_ap, t1[:], r0[:], b_ap,
                                         ALU.mult, ALU.add)
                return t1, r0

            LNE = [nc.vector, nc.vector]

            # LN1 both qt (Sqrt ops batched; qt1 runs on gpsimd)
            st1 = []
            for qt in range(2):
                st1.append(ln_stats(xin[qt][:], f"l1q{qt}", LNE[qt]))
            # LN1 apply inlined with the LN2-mean chain interleaved
            # between the two stt ops: sum(h1) = r0*sum(t1) + sum(b1ln),
            # so the reduce runs before h1 even exists.
            st2 = []
            for qt in range(2):
                x_ap, nm, st = st1[qt]
                eng = LNE[qt]
                r0 = sm.tile([128, 1], F32, tag=f"l1q{qt}r0", name="r0")
                nc.vector.reciprocal(r0[:], st[:])
                t1 = sm.tile([128, D], F32, tag=f"l1q{qt}t1", name="t1")
                eng.scalar_tensor_tensor(t1[:], x_ap, nm[:],
                                         lng[:, 0:D], ALU.add, ALU.mult)
                sst1 = sm.tile([128, 1], F32, tag=f"l2s{qt}", name="sst1")
                nc.vector.reduce_sum(sst1[:], t1[:], axis=AX.X)
                tmr = sm.tile([128, 1], F32, tag=f"l2t{qt}", name="tmr")
                nc.vector.tensor_mul(tmr[:], sst1[:], r0[:])
                nm2 = sm.tile([128, 1], F32, tag=f"l2n{qt}", name="nm2")
                nc.vector.tensor_scalar(
                    nm2[:], tmr[:], -1.0 / D, sumb1c,
                    op0=ALU.mult, op1=ALU.subtract)
                eng.scalar_tensor_tensor(h1[qt][:], t1[:], r0[:],
                                         lng[:, D:2 * D],
                                         ALU.mult, ALU.add)
                scr = sm.tile([128, D], F32, tag=f"l2c{qt}", name="scr2")
                vs = sm.tile([128, 1], F32, tag=f"l2v{qt}", name="vs2")
                nc.scalar.activation(scr[:], h1[qt][:], AF.Square,
                                     bias=nm2[:], accum_out=vs[:])
                st = sm.tile([128, 1], F32, tag=f"l2w{qt}", name="st2")
                nc.scalar.activation(st[:], vs[:], AF.Sqrt, bias=epscol,
                                     scale=1.0 / D)
                st2.append((nm2, st))
            for qt in range(2):
                nm2, st = st2[qt]
                ln_apply(h1[qt][:], nm2, st, None, None,
                         fln[qt][:], f"l2q{qt}", LNE[qt])
            # transpose fln -> fT
            for qt in range(2):
                for fc in range(2):
                    tp2 = tr_ps(qt, "trf")
                    nc.tensor.transpose(
                        tp2, fln[qt][:, 128 * fc:128 * fc + 128],
                        idb[:])
                    nc.vector.tensor_copy(
                        fT[fc][:, 128 * qt:128 * qt + 128], tp2)
            # FFN1 + gelu (gelus batched); psum from the now-idle score pool
            for qt in range(2):
                for oc in range(4):
                    fs = ps_sc.tile([128, 4, QS], F32, tag="sc", name="ps1")
                    fp = fs[:, 0, :]
                    for ic in range(2):
                        nc.tensor.matmul(
                            fp[:, 0:128],
                            w1[ic][:, 128 * oc:128 * oc + 128],
                            fT[ic][:, 128 * qt:128 * qt + 128],
                            start=(ic == 0), stop=(ic == 1))
                    nc.scalar.activation(
                        g1T[oc][:, 128 * qt:128 * qt + 128], fp[:, 0:128],
                        AF.Gelu, bias=b1c[oc])
            # FFN2 + transpose back + residual + store (per qt)
            for qt in range(2):
                for oc in range(2):
                    fs = ps_sc.tile([128, 4, QS], F32, tag="sc", name="ps2")
                    fp = fs[:, 0, :]
                    for ic in range(4):
                        nc.tensor.matmul(
                            fp[:, 0:128],
                            w2[ic][:, 128 * oc:128 * oc + 128],
                            g1T[ic][:, 128 * qt:128 * qt + 128],
                            start=(ic == 0), stop=(ic == 3))
                    nc.scalar.activation(
                        y2T[oc][:, 128 * qt:128 * qt + 128], fp[:, 0:128],
                        AF.Identity, bias=b2c[oc])
                for fc in range(2):
                    tp3 = tr_ps(qt, "trn")
                    nc.tensor.transpose(
                        tp3, y2T[fc][:, 128 * qt:128 * qt + 128],
                        idb[:])
                    nc.vector.tensor_add(
                        out_sb[qt][:, 128 * fc:128 * fc + 128],
                        h1[qt][:, 128 * fc:128 * fc + 128], tp3)
                nc.sync.dma_start(d_out.ap()[128 * qt:128 * qt + 128, :],
                                  out_sb[qt][:])

    nc.compile()
    return nc


_CACHE = {}
USE_FR = True


def _get_nc(use_fr=True):
    if use_fr not in _CACHE:
        _CACHE[use_fr] = build_kernel()
    return _CACHE[use_fr]


def kernel(**inputs):
    import ml_dtypes

    bf = ml_dtypes.bfloat16
    f8 = ml_dtypes.float8_e4m3fn

    h = np.asarray(inputs["h"], np.float32)
    edge_attr = np.asarray(inputs["edge_attr"], np.float32)
    edge_index = np.asarray(inputs["edge_index"])
    Wq, bq = np.asarray(inputs["Wq"], np.float32), np.asarray(inputs["bq"], np.float32)
    Wk, bk = np.asarray(inputs["Wk"], np.float32), np.asarray(inputs["bk"], np.float32)
    Wv, bv = np.asarray(inputs["Wv"], np.float32), np.asarray(inputs["bv"], np.float32)
    Wo, bo = np.asarray(inputs["Wo"], np.float32), np.asarray(inputs["bo"], np.float32)
    We, be = np.asarray(inputs["We"], np.float32), np.asarray(inputs["be"], np.float32)
    ln1_g, ln1_b = np.asarray(inputs["ln1_g"], np.float32), np.asarray(inputs["ln1_b"], np.float32)
    fln_g, fln_b = np.asarray(inputs["fln_g"], np.float32), np.asarray(inputs["fln_b"], np.float32)
    W1, b1 = np.asarray(inputs["W1"], np.float32), np.asarray(inputs["b1"], np.float32)
    W2, b2 = np.asarray(inputs["W2"], np.float32), np.asarray(inputs["b2"], np.float32)

    scale = 1.0 / np.sqrt(np.float32(DK))
    eb = edge_attr @ We + be  # (E, H)
    hT = np.ascontiguousarray(h.T)

    cols = np.zeros((128, 16), np.float32)
    cols[:, 0] = bq[0:128] * scale
    cols[:, 1] = bq[128:256] * scale
    cols[:, 2] = bk[0:128]
    cols[:, 3] = bk[128:256]
    b1_eff = b1 + fln_b @ W1
    for i in range(4):
        cols[:, 4 + i] = b1_eff[128 * i:128 * i + 128]
    cols[:, 8] = b2[0:128]
    cols[:, 9] = b2[128:256]
    cols[:, 11] = EPS
    cols[:, 12] = float(ln1_b.sum()) / D

    bo_eff = (bv @ Wo + bo).reshape(1, D)
    lng = np.concatenate(
        [np.tile(x.reshape(1, D), (128, 1))
         for x in [ln1_g, ln1_b, fln_g, fln_b]], axis=1)

    w1_eff = fln_g.reshape(D, 1) * W1
    idbo = np.zeros((128, 384), np.float32)
    idbo[:, 0:128] = np.eye(128)
    idbo[0, 128:384] = bo_eff[0]
    w2lng = np.concatenate(
        [W2[0:128], W2[128:256], W2[256:384], W2[384:512], lng], axis=1)
    common = {
        "hT": hT.astype(bf),
        "w2lng": w2lng.astype(bf),
        "idbo": idbo.astype(bf),
    }
    id8dr = np.stack(
        [np.eye(128, dtype=np.float32),
         np.zeros((128, 128), np.float32)], axis=1).reshape(128, 256)
    wq_s = Wq * scale

    src = edge_index[0].astype(np.int64)
    dst = edge_index[1].astype(np.int64)
    in_maps = []
    for c in range(N_CORES):
        r0 = c * QS
        m = dict(common)
        qp = np.concatenate(
            [hT[:, 0:QS], hT[:, r0:r0 + QS], wq_s, Wk, Wv],
            axis=1).astype(bf)
        colsb = cols.astype(np.float32).view(bf).reshape(128, 32)
        m["qpack"] = np.ascontiguousarray(np.concatenate(
            [qp, np.concatenate([colsb, np.zeros_like(colsb)], axis=0)],
            axis=1))
        hres_c = h[r0:r0 + QS]
        m["epack"] = np.ascontiguousarray(np.concatenate(
            [np.concatenate([hres_c[128 * b:128 * b + 128],
                             Wo[128 * b:128 * b + 128],
                             w1_eff[128 * b:128 * b + 128]], axis=1)
             for b in range(2)], axis=0)).astype(bf)
        sel = (src >= r0) & (src < r0 + QS)
        slab = np.zeros((N, H, QS), np.float32)
        slab[dst[sel], :, src[sel] - r0] = eb[sel]
        slab = slab[:, ORDER, :]  # head slot order
        slab = slab.reshape(NCH, 128, H * QS)
        m["bias8"] = np.ascontiguousarray(slab).astype(f8).reshape(
            NCH, 128, H, QS)
        m["b0pack"] = np.ascontiguousarray(np.concatenate(
            [id8dr, slab[0]], axis=1)).astype(f8)
        in_maps.append(m)

    nc = _get_nc(USE_FR)
    res = run_bass_kernel_spmd(nc, in_maps, core_ids=list(range(N_CORES)))
    out = np.concatenate([res.results[cc]["out"] for cc in range(N_CORES)],
                         axis=0)
    return out.astype(np.float32)
